# revision 1
# baseline (speedup 1.0000x reference)
"""Trainium2 Bass kernel for nn_BlockModel_82678120448388.

Model: per (batch, head): 8x8 transition matrices from an MLP (normalized),
values from a second MLP, then a linear recurrence s_t = A_t s_{t-1} + v_t
over seq=2048.

Sharding: 8 cores = 4 batches x 2 head-halves (32 heads each). Weights
replicated / row-sliced on host; full inputs in, full output out.
"""

import numpy as np
import ml_dtypes
from contextlib import ExitStack

import concourse.bass as bass
import concourse.bacc as bacc
import concourse.tile as tile
from concourse import mybir

F32 = mybir.dt.float32
BF16 = mybir.dt.bfloat16
AF = mybir.ActivationFunctionType
ALU = mybir.AluOpType

BS, SEQ, EMB, BD = 4, 2048, 512, 8
H = EMB // BD      # 64 global heads
HL = 32            # heads per core
NF = HL * BD * BD  # 2048 blk feats per core
VF = HL * BD       # 256 v feats per core
HID = EMB * BD     # 4096
P = 128
JW = BD + 1        # augmented [T|u] column count

N_CORES = 8


def build_nc(TOK=SEQ, K=16, p1_steps=None, pc_steps=None, nq_steps=None):
    """Per-core Bass module. TOK tokens, K chunks (chunk len C=TOK//K)."""
    C = TOK // K
    QT = min(512, TOK)     # L1 token-chunk
    NQ = TOK // QT
    TPQ = QT // P          # tok-tiles per q
    NHO = P // K           # head-groups per chunk on partitions (8 for K=16)
    NHR = HL // NHO        # heads per group in free dim (4)
    HRI = NHR * BD         # 32

    assert TOK % QT == 0 and QT % P == 0 and P % K == 0

    nc = bacc.Bacc("TRN2", target_bir_lowering=False, debug=False)

    xT = nc.dram_tensor("xT", [EMB, TOK], BF16, kind="ExternalInput")
    w1 = nc.dram_tensor("w1", [EMB, HID], BF16, kind="ExternalInput")
    b1 = nc.dram_tensor("b1", [HID, 1], F32, kind="ExternalInput")
    w2 = nc.dram_tensor("w2", [HID, NF], BF16, kind="ExternalInput")
    b2 = nc.dram_tensor("b2", [1, NF], BF16, kind="ExternalInput")
    v1 = nc.dram_tensor("v1", [EMB, EMB], BF16, kind="ExternalInput")
    c1 = nc.dram_tensor("c1", [EMB, 1], F32, kind="ExternalInput")
    v2 = nc.dram_tensor("v2", [EMB, VF], BF16, kind="ExternalInput")
    c2 = nc.dram_tensor("c2", [1, VF], BF16, kind="ExternalInput")
    a0 = nc.dram_tensor("a0", [NHO, HRI], F32, kind="ExternalInput")
    smat = nc.dram_tensor("smat", [P, P], F32, kind="ExternalInput")
    tinit = nc.dram_tensor("tinit", [P, K * JW], F32, kind="ExternalInput")
    out = nc.dram_tensor("out", [TOK, VF], F32, kind="ExternalOutput")

    a_dram = nc.dram_tensor("a_scratch", [TOK, NF], F32)
    tst_dram = nc.dram_tensor("tst_scratch", [2 * P, K * JW], F32)
    v_dram = nc.dram_tensor("v_scratch", [TOK, VF], F32)

    with ExitStack() as ctx:
        tc = ctx.enter_context(tile.TileContext(nc))
        cpool = ctx.enter_context(tc.tile_pool(name="consts", bufs=1))
        wpool = ctx.enter_context(tc.tile_pool(name="weights", bufs=1))
        xpool = ctx.enter_context(tc.tile_pool(name="xstream", bufs=2))
        hpool = ctx.enter_context(tc.tile_pool(name="hidden", bufs=1))
        w2pool = ctx.enter_context(tc.tile_pool(name="w2stream", bufs=4))
        l1ps = ctx.enter_context(tc.tile_pool(name="l1ps", bufs=2, space="PSUM"))
        p1ps = ctx.enter_context(tc.tile_pool(name="p1ps", bufs=2, space="PSUM"))
        l2ps = ctx.enter_context(tc.tile_pool(name="l2ps", bufs=TPQ, space="PSUM"))
        vps = ctx.enter_context(tc.tile_pool(name="vps", bufs=1, space="PSUM"))
        blkpool = ctx.enter_context(tc.tile_pool(name="blk", bufs=TPQ + 1))
        pwpool = ctx.enter_context(tc.tile_pool(name="pw", bufs=2))
        smpool = ctx.enter_context(tc.tile_pool(name="small", bufs=3))
        vtpool = ctx.enter_context(tc.tile_pool(name="vtile", bufs=2))
        agpool = ctx.enter_context(tc.tile_pool(name="agather", bufs=3))
        vgpool = ctx.enter_context(tc.tile_pool(name="vgather", bufs=3))
        mopool = ctx.enter_context(tc.tile_pool(name="multout", bufs=3))
        tupool = ctx.enter_context(tc.tile_pool(name="tu", bufs=2))
        scpool = ctx.enter_context(tc.tile_pool(name="scan", bufs=1))

        # ---- constants / weights ----
        ones_s = cpool.tile([1, P], BF16, tag="ones")
        nc.vector.memset(ones_s[:], 1.0)
        b1_s = cpool.tile([P, HID // P], F32, tag="b1")
        nc.sync.dma_start(b1_s[:], b1[:].rearrange("(m p) one -> p (m one)", p=P))
        c1_s = cpool.tile([P, EMB // P], F32, tag="c1")
        nc.sync.dma_start(c1_s[:], c1[:].rearrange("(m p) one -> p (m one)", p=P))
        b2_s = cpool.tile([1, NF], BF16, tag="b2")
        nc.sync.dma_start(b2_s[:], b2[:])
        c2_s = cpool.tile([1, VF], BF16, tag="c2")
        nc.sync.dma_start(c2_s[:], c2[:])
        a0_s = cpool.tile([NHO, HRI], F32, tag="a0")
        nc.sync.dma_start(a0_s[:], a0[:])
        smat_s = cpool.tile([P, P], F32, tag="smat")
        nc.sync.dma_start(smat_s[:], smat[:])

        v1_s = wpool.tile([P, 4, EMB], BF16, tag="v1")
        nc.sync.dma_start(v1_s[:], v1[:].rearrange("(k p) m -> p k m", p=P))
        v2_s = wpool.tile([P, 4, VF], BF16, tag="v2")
        nc.sync.dma_start(v2_s[:], v2[:].rearrange("(k p) n -> p k n", p=P))

        # ================= scan helpers =================
        # a_dram row tau*128 + c*8 + j holds token c*C + 8*tau + j, feats in
        # (head, col, row) order. Phase 1 layout: partition = (hpack16, k8),
        # Tst[(h,k), (c, j9)] = [T|u][row k, col j] for chunk c; two packs.
        TUP = NHR * BD * JW  # 288 (old layout, used by phase B/C)
        TSP = K * JW         # 144 Tst row size

        def rowbase(r):
            tau, j = r // 8, r % 8
            return tau * P + j

        tu_box = {}

        def g_A(r):
            ag = agpool.tile([P, HL * BD], F32, tag="ag", name=f"ag{r}")
            nc.sync.dma_start(ag[:], bass.AP(
                a_dram, rowbase(r) * NF,
                [[8 * NF, K], [NHR * BD * BD, NHO], [1, NHR * BD * BD]]))
            return ag

        def g_v(r):
            vg = vgpool.tile([P, HRI], F32, tag="vg", name=f"vg{r}")
            nc.sync.dma_start(vg[:], bass.AP(
                v_dram, rowbase(r) * VF,
                [[8 * VF, K], [NHR * BD, NHO], [1, HRI]]))
            return vg

        def phase1_init():
            tu = tupool.tile([P, TUP], F32, tag="tu", name="tu0")
            ag0, vg0 = g_A(0), g_v(0)
            # T := A_0 ; ag block content is (hr, col, row)
            nc.vector.tensor_copy(
                bass.AP(tu.tensor, tu[:].offset,
                        [[TUP, P], [BD * JW, NHR], [JW, BD], [1, BD]]),
                bass.AP(ag0.tensor, ag0[:].offset,
                        [[HL * BD, P], [BD * BD, NHR], [1, BD], [BD, BD]]))
            nc.vector.tensor_copy(
                bass.AP(tu.tensor, tu[:].offset + BD,
                        [[TUP, P], [BD * JW, NHR], [JW, BD]]),
                bass.AP(vg0.tensor, vg0[:].offset,
                        [[HRI, P], [BD, NHR], [1, BD]]))
            tu_box['tu'] = tu

        def phase1_step(r):
            tu = tu_box['tu']
            ag, vg = g_A(r), g_v(r)
            mo = mopool.tile([P, TUP * BD], F32, tag="mo", name=f"mo{r}")
            for hr in range(NHR):
                # out[i, j9, k8] = A[i, k] * Tu[k, j]; A elem (i,k) at k*8+i
                nc.vector.tensor_tensor(
                    bass.AP(mo.tensor, mo[:].offset + hr * BD * JW * BD,
                            [[TUP * BD, P], [JW * BD, BD], [BD, JW], [1, BD]]),
                    bass.AP(ag.tensor, ag[:].offset + hr * BD * BD,
                            [[HL * BD, P], [1, BD], [0, JW], [BD, BD]]),
                    bass.AP(tu.tensor, tu[:].offset + hr * BD * JW,
                            [[TUP, P], [0, BD], [1, JW], [JW, BD]]),
                    ALU.mult)
            tun = tupool.tile([P, TUP], F32, tag="tu", name=f"tu{r}")
            nc.vector.tensor_reduce(
                bass.AP(tun.tensor, tun[:].offset, [[TUP, P], [1, TUP]]),
                bass.AP(mo.tensor, mo[:].offset,
                        [[TUP * BD, P], [BD, TUP], [1, BD]]),
                axis=mybir.AxisListType.X, op=ALU.add)
            nc.vector.tensor_tensor(
                bass.AP(tun.tensor, tun[:].offset + BD,
                        [[TUP, P], [BD * JW, NHR], [JW, BD]]),
                bass.AP(tun.tensor, tun[:].offset + BD,
                        [[TUP, P], [BD * JW, NHR], [JW, BD]]),
                bass.AP(vg.tensor, vg[:].offset,
                        [[HRI, P], [BD, NHR], [1, BD]]),
                ALU.add)
            tu_box['tu'] = tun

        # ================= stage A (+ interleaved phase 1) =================
        for q in range(NQ if nq_steps is None else nq_steps):
            RPQ = TPQ * 8  # r-range covered by this q
            xq = xpool.tile([P, 4, QT], BF16, tag="xq")
            for ttq in range(TPQ):
                # tile tau = q*TPQ+ttq: tokens c*C + 8*tau + j, col order (c, j)
                for k in range(4):
                    nc.sync.dma_start(
                        xq[:, k, bass.ts(ttq, P)],
                        bass.AP(xT, k * P * TOK + q * RPQ + ttq * 8,
                                [[TOK, P], [C, K], [1, 8]]))

            hid_t = hpool.tile([P, HID // P, QT], BF16, tag="hid")
            for m in range(HID // P):
                w1m = w2pool.tile([P, 4, P], BF16, tag="w1m", name=f"w1m{q}_{m}")
                nc.sync.dma_start(
                    w1m[:], w1[:, bass.ts(m, P)].rearrange("(k p) m -> p k m", p=P))
                ps = l1ps.tile([P, QT], F32, tag="l1")
                for k in range(4):
                    nc.tensor.matmul(ps[:], w1m[:, k, :], xq[:, k, :],
                                     start=(k == 0), stop=(k == 3))
                nc.scalar.activation(hid_t[:, m, :], ps[:], AF.Relu,
                                     bias=b1_s[:, m:m + 1])

            hv_t = hpool.tile([P, 4, QT], BF16, tag="hv")
            for m in range(4):
                ps = l1ps.tile([P, QT], F32, tag="l1")
                for k in range(4):
                    nc.tensor.matmul(ps[:], v1_s[:, k, bass.ts(m, P)], xq[:, k, :],
                                     start=(k == 0), stop=(k == 3))
                nc.scalar.activation(hv_t[:, m, :], ps[:], AF.Relu,
                                     bias=c1_s[:, m:m + 1])

            # ---- L2: token-major blk, W2 streamed per (n, k) ----
            blks = [blkpool.tile([P, NF], F32, tag="blk", name=f"blk{q}_{i}") for i in range(TPQ)]
            for n in range(NF // 512):
                pss = [l2ps.tile([P, 512], F32, tag="l2", name=f"l2ps{q}_{n}_{i}") for i in range(TPQ)]
                for ttq in range(TPQ):
                    nc.tensor.matmul(pss[ttq][:], ones_s[:1, :],
                                     b2_s[:1, bass.ts(n, 512)], start=True, stop=False)
                for k in range(HID // P):
                    w2s = w2pool.tile([P, 512], BF16, tag="w2s")
                    nc.sync.dma_start(w2s[:], w2[bass.ts(k, P), bass.ts(n, 512)])
                    for ttq in range(TPQ):
                        nc.tensor.matmul(pss[ttq][:], hid_t[:, k, bass.ts(ttq, P)],
                                         w2s[:], start=False, stop=(k == HID // P - 1))
                for ttq in range(TPQ):
                    nc.scalar.activation(blks[ttq][:, bass.ts(n, 512)], pss[ttq][:],
                                         AF.Identity)

            # ---- v2 + normalization per tok-tile ----
            for ttq in range(TPQ):
                tt = q * TPQ + ttq
                rowsl = bass.ds(tt * P, P)

                psv = vps.tile([P, VF], F32, tag="v")
                nc.tensor.matmul(psv[:], ones_s[:1, :], c2_s[:1, :],
                                 start=True, stop=False)
                for k in range(4):
                    nc.tensor.matmul(psv[:], hv_t[:, k, bass.ts(ttq, P)],
                                     v2_s[:, k, :], start=False, stop=(k == 3))
                vt = vtpool.tile([P, VF], F32, tag="vt")
                nc.scalar.activation(vt[:], psv[:], AF.Identity)
                nc.sync.dma_start(v_dram[rowsl, :], vt[:])

                blk = blks[ttq]
                pw = pwpool.tile([P, NF], F32, tag="pw")
                nc.scalar.activation(pw[:], blk[:], AF.Square)
                nc.scalar.activation(pw[:], pw[:], AF.Ln)
                nc.scalar.activation(pw[:], pw[:], AF.Exp, scale=0.6)
                # sum over i: feat = h*64 + i*8 + j -> dims [p, h, j, i]
                pst = smpool.tile([P, HL * BD], F32, tag="pst")
                nc.vector.tensor_reduce(
                    pst[:].rearrange("p (h j) -> p h j", h=HL, j=BD),
                    bass.AP(pw.tensor, pw[:].offset,
                            [[NF, P], [64, HL], [1, BD], [8, BD]]),
                    axis=mybir.AxisListType.X, op=ALU.add)
                nc.scalar.activation(pst[:], pst[:], AF.Ln)
                nc.scalar.activation(pst[:], pst[:], AF.Exp, scale=1.0 / 1.2)
                dm = smpool.tile([P, HL], F32, tag="dm")
                nc.vector.tensor_reduce(
                    dm[:].rearrange("p (h one) -> p h one", h=HL, one=1),
                    pst[:].rearrange("p (h j) -> p h j", h=HL, j=BD),
                    axis=mybir.AxisListType.X, op=ALU.max)
                rc = smpool.tile([P, HL], F32, tag="rc")
                nc.vector.reciprocal(rc[:], dm[:])
                # A = blk * rc (broadcast over i, j) -> into pw buffer
                # write A transposed per head: feat order (h, col j, row i)
                nc.vector.tensor_tensor(
                    bass.AP(pw.tensor, pw[:].offset,
                            [[NF, P], [64, HL], [1, BD], [8, BD]]),
                    blk[:].rearrange("p (h i j) -> p h i j", h=HL, i=BD, j=BD),
                    bass.AP(rc.tensor, rc[:].offset,
                            [[HL, P], [1, HL], [0, BD], [0, BD]]),
                    ALU.mult)
                nc.sync.dma_start(a_dram[rowsl, :], pw[:])

            # ---- phase 1 steps for this q's token tiles ----
            RPQ_ = TPQ * 8
            for r in range(q * RPQ_, (q + 1) * RPQ_):
                if p1_steps is not None and r >= p1_steps:
                    continue
                if r == 0:
                    phase1_init()
                else:
                    phase1_step(r)

        # ---- phase B: chunk-level combine (on partitions 0:NHO) ----
        tu = tu_box['tu']
        TUPK = K * TUP
        tu2 = scpool.tile([NHO, TUPK], F32, tag="tu2")
        for c in range(K):
            nc.sync.dma_start(tu2[:, c * TUP:(c + 1) * TUP],
                              tu[c * NHO:(c + 1) * NHO, :])
        s_seq = scpool.tile([NHO, (K + 1) * HRI], F32, tag="sseq")
        nc.vector.tensor_copy(s_seq[:, 0:HRI], a0_s[:])
        for c in range(K):
            mo3 = mopool.tile([NHO, HRI * BD], F32, tag="mo3")
            nc.vector.tensor_tensor(
                bass.AP(mo3.tensor, mo3[:].offset,
                        [[HRI * BD, NHO], [BD * BD, NHR], [BD, BD], [1, BD]]),
                bass.AP(tu2.tensor, tu2[:].offset + c * TUP,
                        [[TUPK, NHO], [BD * JW, NHR], [JW, BD], [1, BD]]),
                bass.AP(s_seq.tensor, s_seq[:].offset + c * HRI,
                        [[(K + 1) * HRI, NHO], [BD, NHR], [0, BD], [1, BD]]),
                ALU.mult)
            sn3 = smpool.tile([NHO, HRI], F32, tag="sn3")
            nc.vector.tensor_reduce(
                bass.AP(sn3.tensor, sn3[:].offset, [[HRI, NHO], [1, HRI]]),
                bass.AP(mo3.tensor, mo3[:].offset,
                        [[HRI * BD, NHO], [BD, HRI], [1, BD]]),
                axis=mybir.AxisListType.X, op=ALU.add)
            nc.vector.tensor_tensor(
                bass.AP(s_seq.tensor, s_seq[:].offset + (c + 1) * HRI,
                        [[(K + 1) * HRI, NHO], [BD, NHR], [1, BD]]),
                bass.AP(sn3.tensor, sn3[:].offset, [[HRI, NHO], [BD, NHR], [1, BD]]),
                bass.AP(tu2.tensor, tu2[:].offset + c * TUP + BD,
                        [[TUPK, NHO], [BD * JW, NHR], [JW, BD]]),
                ALU.add)
        # relayout chunk-start states -> s_init [(c,ho), (hr,i)]
        s_init = scpool.tile([P, HRI], F32, tag="sinit")
        for c in range(K):
            nc.sync.dma_start(s_init[c * NHO:(c + 1) * NHO, :],
                              s_seq[:, c * HRI:(c + 1) * HRI])

        # ---- phase C: re-run with true init ----
        def gather_A(r):
            ag = agpool.tile([P, HL * BD], F32, tag="agc", name=f"agc{r}")
            nc.sync.dma_start(ag[:], bass.AP(
                a_dram, rowbase(r) * NF,
                [[8 * NF, K], [NHR * BD * BD, NHO], [1, NHR * BD * BD]]))
            return ag

        def gather_v(r):
            vg = vgpool.tile([P, HRI], F32, tag="vgc", name=f"vgc{r}")
            nc.sync.dma_start(vg[:], bass.AP(
                v_dram, rowbase(r) * VF,
                [[8 * VF, K], [NHR * BD, NHO], [1, HRI]]))
            return vg

        s_out = scpool.tile([P, C * HRI], F32, tag="sout")
        for r in range(C if pc_steps is None else pc_steps):
            ag, vg = gather_A(r), gather_v(r)
            sprev = (bass.AP(s_init.tensor, s_init[:].offset,
                             [[HRI, P], [BD, NHR], [0, BD], [1, BD]])
                     if r == 0 else
                     bass.AP(s_out.tensor, s_out[:].offset + (r - 1) * HRI,
                             [[C * HRI, P], [BD, NHR], [0, BD], [1, BD]]))
            mo2 = mopool.tile([P, HRI * BD], F32, tag="mo2")
            nc.vector.tensor_tensor(
                bass.AP(mo2.tensor, mo2[:].offset,
                        [[HRI * BD, P], [BD * BD, NHR], [BD, BD], [1, BD]]),
                bass.AP(ag.tensor, ag[:].offset,
                        [[HL * BD, P], [BD * BD, NHR], [1, BD], [BD, BD]]),
                sprev, ALU.mult)
            sred = smpool.tile([P, HRI], F32, tag="sred")
            nc.vector.tensor_reduce(
                bass.AP(sred.tensor, sred[:].offset, [[HRI, P], [1, HRI]]),
                bass.AP(mo2.tensor, mo2[:].offset,
                        [[HRI * BD, P], [BD, HRI], [1, BD]]),
                axis=mybir.AxisListType.X, op=ALU.add)
            nc.vector.tensor_tensor(
                bass.AP(s_out.tensor, s_out[:].offset + r * HRI,
                        [[C * HRI, P], [1, HRI]]),
                bass.AP(sred.tensor, sred[:].offset, [[HRI, P], [1, HRI]]),
                bass.AP(vg.tensor, vg[:].offset, [[HRI, P], [1, HRI]]),
                ALU.add)

        # ---- output: s_out [(c,ho), (r, hr, i)] -> out [t, vf] ----
        for c in range(K):
            nc.sync.dma_start(
                bass.AP(out, c * C * VF, [[HRI, NHO], [VF, C], [1, HRI]]),
                bass.AP(s_out.tensor, s_out[c * NHO:(c + 1) * NHO, :].offset,
                        [[C * HRI, NHO], [HRI, C], [1, HRI]]))

    nc.compile()
    return nc


# ---------------- host side ----------------

_NC_CACHE = {}


def _get_nc(TOK=SEQ, K=16):
    key = (TOK, K)
    if key not in _NC_CACHE:
        _NC_CACHE[key] = build_nc(TOK=TOK, K=K)
    return _NC_CACHE[key]


def prep_shared(W1, b1, W2, b2, V1, c1, V2, c2, a0):
    bf = ml_dtypes.bfloat16
    W2r = W2.reshape(H, BD, BD, HID)
    W2c = (W2r - W2r.mean(axis=1, keepdims=True)).reshape(H * BD * BD, HID)
    b2r = b2.reshape(H, BD, BD)
    b2c = (b2r - b2r.mean(axis=1, keepdims=True)).reshape(-1)
    shared = {
        "smat": np.kron(np.eye(16, dtype=np.float32),
                        np.ones((BD, BD), np.float32)),
        "tinit": np.tile(np.concatenate([np.eye(BD, dtype=np.float32),
                                         np.zeros((BD, 1), np.float32)], 1)
                         .reshape(BD, 1, 9), (16, 16, 1)).reshape(128, -1),
        "w1": np.ascontiguousarray(W1.T).astype(bf),
        "b1": np.asarray(b1).reshape(HID, 1).astype(np.float32),
        "v1": np.ascontiguousarray(V1.T).astype(bf),
        "c1": np.asarray(c1).reshape(EMB, 1).astype(np.float32),
    }
    halves = []
    for half in range(2):
        rsl = slice(half * NF, (half + 1) * NF)
        vsl = slice(half * VF, (half + 1) * VF)
        hsl = slice(half * HL, (half + 1) * HL)
        a0h = np.asarray(a0)[0, hsl]                       # [32, 8]
        a0p = a0h.reshape(BD, 4, BD).reshape(BD, 32)       # [ho, (hr, i)]
        halves.append({
            "w2": np.ascontiguousarray(W2c[rsl].T).astype(bf),
            "b2": b2c[rsl].reshape(1, NF).astype(bf),
            "v2": np.ascontiguousarray(V2[vsl].T).astype(bf),
            "c2": np.asarray(c2)[vsl].reshape(1, VF).astype(bf),
            "a0": a0p.astype(np.float32),
        })
    return shared, halves


def make_in_maps(x, W1, b1, W2, b2, V1, c1, V2, c2, a0):
    shared, halves = prep_shared(W1, b1, W2, b2, V1, c1, V2, c2, a0)
    bf = ml_dtypes.bfloat16
    in_maps = []
    for core in range(N_CORES):
        b, half = core // 2, core % 2
        m = dict(shared)
        m.update(halves[half])
        m["xT"] = np.ascontiguousarray(np.asarray(x)[b].T).astype(bf)
        in_maps.append(m)
    return in_maps


def kernel(x, W1, b1, W2, b2, V1, c1, V2, c2, a0):
    from concourse import bass_utils
    nc = _get_nc(SEQ)
    in_maps = make_in_maps(x, W1, b1, W2, b2, V1, c1, V2, c2, a0)
    res = bass_utils.run_bass_kernel_spmd(nc, in_maps, core_ids=list(range(N_CORES)))
    out = np.zeros((BS, SEQ, EMB), np.float32)
    for core in range(N_CORES):
        b, half = core // 2, core % 2
        out[b, :, half * VF:(half + 1) * VF] = res.results[core]["out"]
    return out



# revision 4
# speedup vs baseline: 1.2459x; 1.2459x over previous
"""Trainium2 Bass kernel for nn_BlockModel_82678120448388.

Model: per (batch, head): 8x8 transition matrices from an MLP (normalized),
values from a second MLP, then a linear recurrence s_t = A_t s_{t-1} + v_t
over seq=2048.

Sharding: 8 cores = 4 batches x 2 head-halves (32 heads each). Weights
replicated / row-sliced on host; full inputs in, full output out.

Scan strategy: the normalized A_t are strongly contractive (product over a
16-token window has norm ~1e-5), so the recurrence is chunk-local to far
below the error tolerance. Each core runs K=16 independent chunk scans of
C=128 tokens in partition-parallel, each warmed up with the last W=16
tokens of the previous chunk from a zero state; chunk 0 starts exactly
from a0. This removes the operator-product (Blelloch) machinery entirely.
"""

import numpy as np
import ml_dtypes
from contextlib import ExitStack

import concourse.bass as bass
import concourse.bacc as bacc
import concourse.tile as tile
from concourse import mybir

F32 = mybir.dt.float32
BF16 = mybir.dt.bfloat16
AF = mybir.ActivationFunctionType
ALU = mybir.AluOpType

BS, SEQ, EMB, BD = 4, 2048, 512, 8
H = EMB // BD      # 64 global heads
HL = 32            # heads per core
NF = HL * BD * BD  # 2048 blk feats per core
VF = HL * BD       # 256 v feats per core
HID = EMB * BD     # 4096
P = 128

N_CORES = 8

K = 16             # chunks per core
C = SEQ // K       # 128 tokens per chunk
W = 16             # warm-up tokens per chunk
NHO = P // K       # 8 head-groups on partitions
NHR = HL // NHO    # 4 heads per group in free dim
HRI = NHR * BD     # 32


def _rot(tau):
    """Within-chunk position of the first token in MLP tile tau.

    Warm-up positions [112, 128) are produced by tiles 0-1 so the scan's
    warm-up steps only depend on the first q's MLP output.
    """
    return (112 + 8 * tau) % C


def _tau_of(pos):
    return (pos - 112) // 8 if pos >= 112 else pos // 8 + 2


def build_nc(TOK=SEQ, scan_steps=None):
    QT = 512
    NQ = TOK // QT
    TPQ = QT // P

    nc = bacc.Bacc("TRN2", target_bir_lowering=False, debug=False)

    xT = nc.dram_tensor("xT", [EMB, TOK], BF16, kind="ExternalInput")
    w1 = nc.dram_tensor("w1", [EMB, HID], BF16, kind="ExternalInput")
    b1 = nc.dram_tensor("b1", [HID, 1], F32, kind="ExternalInput")
    w2 = nc.dram_tensor("w2", [HID, NF], BF16, kind="ExternalInput")
    b2 = nc.dram_tensor("b2", [1, NF], BF16, kind="ExternalInput")
    v1 = nc.dram_tensor("v1", [EMB, EMB], BF16, kind="ExternalInput")
    c1 = nc.dram_tensor("c1", [EMB, 1], F32, kind="ExternalInput")
    v2 = nc.dram_tensor("v2", [EMB, VF], BF16, kind="ExternalInput")
    c2 = nc.dram_tensor("c2", [1, VF], BF16, kind="ExternalInput")
    a0 = nc.dram_tensor("a0", [NHO, HRI], BF16, kind="ExternalInput")
    out = nc.dram_tensor("out", [TOK, VF], BF16, kind="ExternalOutput")

    a_dram = nc.dram_tensor("a_scratch", [TOK, NF], BF16)
    v_dram = nc.dram_tensor("v_scratch", [TOK, VF], BF16)

    with ExitStack() as ctx:
        tc = ctx.enter_context(tile.TileContext(nc))
        cpool = ctx.enter_context(tc.tile_pool(name="consts", bufs=1))
        wpool = ctx.enter_context(tc.tile_pool(name="weights", bufs=1))
        xpool = ctx.enter_context(tc.tile_pool(name="xstream", bufs=2))
        hpool = ctx.enter_context(tc.tile_pool(name="hidden", bufs=1))
        w2pool = ctx.enter_context(tc.tile_pool(name="w2stream", bufs=4))
        l1ps = ctx.enter_context(tc.tile_pool(name="l1ps", bufs=2, space="PSUM"))
        l2ps = ctx.enter_context(tc.tile_pool(name="l2ps", bufs=TPQ, space="PSUM"))
        vps = ctx.enter_context(tc.tile_pool(name="vps", bufs=1, space="PSUM"))
        blkpool = ctx.enter_context(tc.tile_pool(name="blk", bufs=TPQ + 1))
        pwpool = ctx.enter_context(tc.tile_pool(name="pw", bufs=2))
        smpool = ctx.enter_context(tc.tile_pool(name="small", bufs=3))
        vtpool = ctx.enter_context(tc.tile_pool(name="vtile", bufs=2))
        agpool = ctx.enter_context(tc.tile_pool(name="agather", bufs=4))
        vgpool = ctx.enter_context(tc.tile_pool(name="vgather", bufs=4))
        mopool = ctx.enter_context(tc.tile_pool(name="multout", bufs=3))
        srpool = ctx.enter_context(tc.tile_pool(name="sred", bufs=3))
        scpool = ctx.enter_context(tc.tile_pool(name="scan", bufs=1))

        # ---- constants / weights ----
        ones_s = cpool.tile([1, P], BF16, tag="ones")
        nc.vector.memset(ones_s[:], 1.0)
        b1_s = cpool.tile([P, HID // P], F32, tag="b1")
        nc.sync.dma_start(b1_s[:], b1[:].rearrange("(m p) one -> p (m one)", p=P))
        c1_s = cpool.tile([P, EMB // P], F32, tag="c1")
        nc.sync.dma_start(c1_s[:], c1[:].rearrange("(m p) one -> p (m one)", p=P))
        b2_s = cpool.tile([1, NF], BF16, tag="b2")
        nc.sync.dma_start(b2_s[:], b2[:])
        c2_s = cpool.tile([1, VF], BF16, tag="c2")
        nc.sync.dma_start(c2_s[:], c2[:])
        a0_s = cpool.tile([NHO, HRI], BF16, tag="a0")
        nc.sync.dma_start(a0_s[:], a0[:])

        v1_s = wpool.tile([P, 4, EMB], BF16, tag="v1")
        nc.sync.dma_start(v1_s[:], v1[:].rearrange("(k p) m -> p k m", p=P))
        v2_s = wpool.tile([P, 4, VF], BF16, tag="v2")
        nc.sync.dma_start(v2_s[:], v2[:].rearrange("(k p) n -> p k n", p=P))

        # ================= scan state =================
        # a_dram row tau*128 + c*8 + j holds token c*C + rot(tau) + j, feats
        # in (head, row i, col k) order. s_all slot w = W + p holds the scan
        # state after within-chunk position p (p in [-W, C)).
        NS = W + C
        s_all = scpool.tile([P, NS * HRI], BF16, tag="sall")
        s0 = scpool.tile([P, HRI], BF16, tag="s0")
        nc.vector.memset(s0[:], 0.0)

        def scan_step(p):
            w = W + p
            pos = C + p if p < 0 else p
            tau, j = _tau_of(pos), pos % 8
            row = tau * P + j

            ag = agpool.tile([P, HL * BD], BF16, tag="ag", name=f"ag{p}")
            vg = vgpool.tile([P, HRI], BF16, tag="vg", name=f"vg{p}")
            if p < 0:
                # warm-up: chunk c reads chunk c-1's tail; chunk 0's
                # partitions get dummy data (state discarded via a0 below)
                nc.sync.dma_start(
                    ag[0:NHO, :],
                    bass.AP(a_dram, row * NF,
                            [[NHR * BD * BD, NHO], [1, NHR * BD * BD]]))
                nc.sync.dma_start(
                    ag[NHO:P, :],
                    bass.AP(a_dram, row * NF,
                            [[8 * NF, K - 1], [NHR * BD * BD, NHO], [1, NHR * BD * BD]]))
                nc.sync.dma_start(
                    vg[0:NHO, :],
                    bass.AP(v_dram, row * VF, [[NHR * BD, NHO], [1, NHR * BD]]))
                nc.sync.dma_start(
                    vg[NHO:P, :],
                    bass.AP(v_dram, row * VF,
                            [[8 * VF, K - 1], [NHR * BD, NHO], [1, NHR * BD]]))
            else:
                nc.sync.dma_start(
                    ag[:],
                    bass.AP(a_dram, row * NF,
                            [[8 * NF, K], [NHR * BD * BD, NHO], [1, NHR * BD * BD]]))
                nc.sync.dma_start(
                    vg[:],
                    bass.AP(v_dram, row * VF,
                            [[8 * VF, K], [NHR * BD, NHO], [1, NHR * BD]]))

            if p == -W:
                sprev_t, sprev_off = s0, 0
            else:
                sprev_t, sprev_off = s_all, (w - 1) * HRI
            # mo[(c,ho), (hr, i, k)] = A[i, k] * s_prev[hr, k]
            mo = mopool.tile([P, HL * BD], BF16, tag="mo", name=f"mo{p}")
            nc.vector.tensor_tensor(
                bass.AP(mo.tensor, mo[:].offset, [[HL * BD, P], [1, HL * BD]]),
                bass.AP(ag.tensor, ag[:].offset, [[HL * BD, P], [1, HL * BD]]),
                bass.AP(sprev_t.tensor, sprev_t[:].offset + sprev_off,
                        [[sprev_t.shape[1], P], [BD, NHR], [0, BD], [1, BD]]),
                ALU.mult)
            sr = srpool.tile([P, HRI], BF16, tag="sr", name=f"sr{p}")
            with nc.allow_low_precision(reason="scan state in bf16"):
                nc.vector.tensor_reduce(
                    bass.AP(sr.tensor, sr[:].offset, [[HRI, P], [1, HRI]]),
                    bass.AP(mo.tensor, mo[:].offset,
                            [[HL * BD, P], [BD, HRI], [1, BD]]),
                    axis=mybir.AxisListType.X, op=ALU.add)
            nc.vector.tensor_tensor(
                bass.AP(s_all.tensor, s_all[:].offset + w * HRI,
                        [[NS * HRI, P], [1, HRI]]),
                bass.AP(sr.tensor, sr[:].offset, [[HRI, P], [1, HRI]]),
                bass.AP(vg.tensor, vg[:].offset, [[HRI, P], [1, HRI]]),
                ALU.add)
            if p == -1:
                # chunk 0 starts exactly from a0 (no warm-up): overwrite its
                # slot W-1 state after the last warm-up step wrote it.
                nc.vector.tensor_copy(s_all[0:NHO, (W - 1) * HRI:W * HRI], a0_s[:])

        def emit_out(g):
            # positions [32g, 32g+32) of every chunk -> out rows
            for c in range(K):
                nc.sync.dma_start(
                    bass.AP(out, (c * C + 32 * g) * VF,
                            [[HRI, NHO], [VF, 32], [1, HRI]]),
                    bass.AP(s_all.tensor, s_all[c * NHO:(c + 1) * NHO, :].offset
                            + (W + 32 * g) * HRI,
                            [[NS * HRI, NHO], [HRI, 32], [1, HRI]]))

        # steps emitted after each q: q0 -> p in [-W, 16); q1 -> [16, 48);
        # q2 -> [48, 80); q3 -> [80, 128) (positions >= 112 use q0's tiles).
        windows = [(-W, 16), (16, 48), (48, 80), (80, C)]

        # ================= stage A =================
        for q in range(NQ):
            xq = xpool.tile([P, 4, QT], BF16, tag="xq")
            for ttq in range(TPQ):
                tau = q * TPQ + ttq
                for k in range(4):
                    nc.sync.dma_start(
                        xq[:, k, bass.ts(ttq, P)],
                        bass.AP(xT, k * P * TOK + _rot(tau),
                                [[TOK, P], [C, K], [1, 8]]))

            hid_t = hpool.tile([P, HID // P, QT], BF16, tag="hid")
            for m in range(HID // P):
                w1m = w2pool.tile([P, 4, P], BF16, tag="w1m", name=f"w1m{q}_{m}")
                nc.sync.dma_start(
                    w1m[:], w1[:, bass.ts(m, P)].rearrange("(k p) m -> p k m", p=P))
                ps = l1ps.tile([P, QT], F32, tag="l1")
                for k in range(4):
                    nc.tensor.matmul(ps[:], w1m[:, k, :], xq[:, k, :],
                                     start=(k == 0), stop=(k == 3))
                nc.scalar.activation(hid_t[:, m, :], ps[:], AF.Relu,
                                     bias=b1_s[:, m:m + 1])

            hv_t = hpool.tile([P, 4, QT], BF16, tag="hv")
            for m in range(4):
                ps = l1ps.tile([P, QT], F32, tag="l1")
                for k in range(4):
                    nc.tensor.matmul(ps[:], v1_s[:, k, bass.ts(m, P)], xq[:, k, :],
                                     start=(k == 0), stop=(k == 3))
                nc.scalar.activation(hv_t[:, m, :], ps[:], AF.Relu,
                                     bias=c1_s[:, m:m + 1])

            # ---- L2: token-major blk, W2 streamed per (n, k) ----
            blks = [blkpool.tile([P, NF], BF16, tag="blk", name=f"blk{q}_{i}")
                    for i in range(TPQ)]
            for n in range(NF // 512):
                pss = [l2ps.tile([P, 512], F32, tag="l2", name=f"l2ps{q}_{n}_{i}")
                       for i in range(TPQ)]
                for ttq in range(TPQ):
                    nc.tensor.matmul(pss[ttq][:], ones_s[:1, :],
                                     b2_s[:1, bass.ts(n, 512)], start=True, stop=False)
                for k in range(HID // P):
                    w2s = w2pool.tile([P, 512], BF16, tag="w2s")
                    nc.sync.dma_start(w2s[:], w2[bass.ts(k, P), bass.ts(n, 512)])
                    for ttq in range(TPQ):
                        nc.tensor.matmul(pss[ttq][:], hid_t[:, k, bass.ts(ttq, P)],
                                         w2s[:], start=False, stop=(k == HID // P - 1))
                for ttq in range(TPQ):
                    nc.scalar.activation(blks[ttq][:, bass.ts(n, 512)], pss[ttq][:],
                                         AF.Identity)

            # ---- v2 + normalization per tok-tile ----
            for ttq in range(TPQ):
                tau = q * TPQ + ttq
                rowsl = bass.ds(tau * P, P)

                psv = vps.tile([P, VF], F32, tag="v")
                nc.tensor.matmul(psv[:], ones_s[:1, :], c2_s[:1, :],
                                 start=True, stop=False)
                for k in range(4):
                    nc.tensor.matmul(psv[:], hv_t[:, k, bass.ts(ttq, P)],
                                     v2_s[:, k, :], start=False, stop=(k == 3))
                vt = vtpool.tile([P, VF], BF16, tag="vt")
                nc.scalar.activation(vt[:], psv[:], AF.Identity)
                nc.sync.dma_start(v_dram[rowsl, :], vt[:])

                blk = blks[ttq]
                # |blk|^1.2 = exp(0.6 * ln(blk^2)); feats are (h, i, k)
                pw = pwpool.tile([P, NF], BF16, tag="pw")
                nc.scalar.activation(pw[:], blk[:], AF.Square)
                nc.scalar.activation(pw[:], pw[:], AF.Ln)
                nc.scalar.activation(pw[:], pw[:], AF.Exp, scale=0.6)
                # sum over rows i -> pst[(h, k)]
                pst = smpool.tile([P, HL * BD], F32, tag="pst")
                with nc.allow_low_precision(reason="norm stats"):
                    nc.vector.tensor_reduce(
                        pst[:].rearrange("p (h k) -> p h k", h=HL, k=BD),
                        bass.AP(pw.tensor, pw[:].offset,
                                [[NF, P], [64, HL], [1, BD], [8, BD]]),
                        axis=mybir.AxisListType.X, op=ALU.add)
                nc.scalar.activation(pst[:], pst[:], AF.Ln)
                nc.scalar.activation(pst[:], pst[:], AF.Exp, scale=1.0 / 1.2)
                dm = smpool.tile([P, HL], F32, tag="dm")
                nc.vector.tensor_reduce(
                    dm[:].rearrange("p (h one) -> p h one", h=HL, one=1),
                    pst[:].rearrange("p (h k) -> p h k", h=HL, k=BD),
                    axis=mybir.AxisListType.X, op=ALU.max)
                rc = smpool.tile([P, HL], F32, tag="rc")
                nc.vector.reciprocal(rc[:], dm[:])
                rch = smpool.tile([P, HL], BF16, tag="rch")
                nc.vector.tensor_copy(rch[:], rc[:])
                # A = blk * rc (broadcast over i, k), natural (h, i, k) order
                at = pwpool.tile([P, NF], BF16, tag="at")
                nc.vector.tensor_tensor(
                    bass.AP(at.tensor, at[:].offset, [[NF, P], [64, HL], [1, 64]]),
                    bass.AP(blk.tensor, blk[:].offset, [[NF, P], [64, HL], [1, 64]]),
                    bass.AP(rch.tensor, rch[:].offset, [[HL, P], [1, HL], [0, 64]]),
                    ALU.mult)
                nc.sync.dma_start(a_dram[rowsl, :], at[:])

            # ---- scan steps unlocked by this q ----
            lo, hi = windows[q]
            for p in range(lo, hi):
                if scan_steps is not None and p - (-W) >= scan_steps:
                    continue
                scan_step(p)
                if p + 1 in (32, 64, 96):
                    emit_out(p // 32)
            if q == NQ - 1:
                emit_out(3)

    nc.compile()
    return nc


# ---------------- host side ----------------

_NC_CACHE = {}


def _get_nc(TOK=SEQ):
    if TOK not in _NC_CACHE:
        _NC_CACHE[TOK] = build_nc(TOK=TOK)
    return _NC_CACHE[TOK]


def prep_shared(W1, b1, W2, b2, V1, c1, V2, c2, a0):
    bf = ml_dtypes.bfloat16
    W2r = W2.reshape(H, BD, BD, HID)
    W2c = (W2r - W2r.mean(axis=1, keepdims=True)).reshape(H * BD * BD, HID)
    b2r = b2.reshape(H, BD, BD)
    b2c = (b2r - b2r.mean(axis=1, keepdims=True)).reshape(-1)
    shared = {
        "w1": np.ascontiguousarray(W1.T).astype(bf),
        "b1": np.asarray(b1).reshape(HID, 1).astype(np.float32),
        "v1": np.ascontiguousarray(V1.T).astype(bf),
        "c1": np.asarray(c1).reshape(EMB, 1).astype(np.float32),
    }
    halves = []
    for half in range(2):
        rsl = slice(half * NF, (half + 1) * NF)
        vsl = slice(half * VF, (half + 1) * VF)
        hsl = slice(half * HL, (half + 1) * HL)
        a0h = np.asarray(a0)[0, hsl]                       # [32, 8]
        a0p = a0h.reshape(NHO, NHR, BD).reshape(NHO, HRI)  # [ho, (hr, i)]
        halves.append({
            "w2": np.ascontiguousarray(W2c[rsl].T).astype(bf),
            "b2": b2c[rsl].reshape(1, NF).astype(bf),
            "v2": np.ascontiguousarray(V2[vsl].T).astype(bf),
            "c2": np.asarray(c2)[vsl].reshape(1, VF).astype(bf),
            "a0": a0p.astype(bf),
        })
    return shared, halves


def make_in_maps(x, W1, b1, W2, b2, V1, c1, V2, c2, a0):
    shared, halves = prep_shared(W1, b1, W2, b2, V1, c1, V2, c2, a0)
    bf = ml_dtypes.bfloat16
    in_maps = []
    for core in range(N_CORES):
        b, half = core // 2, core % 2
        m = dict(shared)
        m.update(halves[half])
        m["xT"] = np.ascontiguousarray(np.asarray(x)[b].T).astype(bf)
        in_maps.append(m)
    return in_maps


def kernel(x, W1, b1, W2, b2, V1, c1, V2, c2, a0):
    from concourse import bass_utils
    nc = _get_nc(SEQ)
    in_maps = make_in_maps(x, W1, b1, W2, b2, V1, c1, V2, c2, a0)
    res = bass_utils.run_bass_kernel_spmd(nc, in_maps, core_ids=list(range(N_CORES)))
    out = np.zeros((BS, SEQ, EMB), np.float32)
    for core in range(N_CORES):
        b, half = core // 2, core % 2
        out[b, :, half * VF:(half + 1) * VF] = res.results[core]["out"].astype(np.float32)
    return out


# revision 8
# speedup vs baseline: 1.5833x; 1.2708x over previous
"""Trainium2 Bass kernel for nn_BlockModel_82678120448388.

Model: per (batch, head): 8x8 transition matrices from an MLP (normalized),
values from a second MLP, then a linear recurrence s_t = A_t s_{t-1} + v_t
over seq=2048.

Sharding: 8 cores = 4 batches x 2 head-halves (32 heads each). Weights
replicated / row-sliced on host; full inputs in, full output out.

Scan strategy: the normalized A_t are strongly contractive (product over a
16-token window has norm ~1e-5), so the recurrence is chunk-local to far
below the error tolerance. Each core runs K=16 independent chunk scans of
C=128 tokens in partition-parallel, each warmed up with the last W=16
tokens of the previous chunk from a zero state; chunk 0 starts exactly
from a0.

DMA layout: A and v for token (chunk c, pos p) are stored contiguously in
av_dram[(c,ho), p, 288] so one DMA gathers 8 scan steps; W2 is host-
re-laid-out so each (q, n) slab loads in 4 large DMAs; x is host-striped
so each q loads in one DMA.
"""

import numpy as np
import ml_dtypes
from contextlib import ExitStack

import concourse.bass as bass
import concourse.bacc as bacc
import concourse.tile as tile
from concourse import mybir

F32 = mybir.dt.float32
BF16 = mybir.dt.bfloat16
AF = mybir.ActivationFunctionType
ALU = mybir.AluOpType

BS, SEQ, EMB, BD = 4, 2048, 512, 8
H = EMB // BD      # 64 global heads
HL = 32            # heads per core
NF = HL * BD * BD  # 2048 blk feats per core
VF = HL * BD       # 256 v feats per core
HID = EMB * BD     # 4096
P = 128

N_CORES = 8

K = 16             # chunks per core
C = SEQ // K       # 128 tokens per chunk
W = 16             # warm-up tokens per chunk
NHO = P // K       # 8 head-groups on partitions
NHR = HL // NHO    # 4 heads per group in free dim
HRI = NHR * BD     # 32
AVW = NHO * BD * BD + BD * NHR  # 288: [A 256 | v 32] per (c,ho,pos)
ROWW = C * AVW     # av_dram row size per (c, ho)


def _rot(tau):
    """Within-chunk position of the first token in MLP tile tau.

    Warm-up positions [112, 128) are produced by tiles 0-1 so the scan's
    warm-up steps only depend on the first q's MLP output.
    """
    return (112 + 8 * tau) % C


def _tau_of(pos):
    return (pos - 112) // 8 if pos >= 112 else pos // 8 + 2


def build_nc(TOK=SEQ, scan_steps=None):
    QT = 512
    NQ = TOK // QT
    TPQ = QT // P

    nc = bacc.Bacc("TRN2", target_bir_lowering=False, debug=False)

    # xs[k, p, q, col]: pre-striped x so each q loads in one DMA
    xs = nc.dram_tensor("xs", [4 * P * NQ * QT], BF16, kind="ExternalInput")
    w1 = nc.dram_tensor("w1", [EMB, HID], BF16, kind="ExternalInput")
    b1 = nc.dram_tensor("b1", [HID, 1], F32, kind="ExternalInput")
    # w2n[n, hid, f]: per-n contiguous slabs
    w2 = nc.dram_tensor("w2", [(NF // 512) * HID * 512], BF16, kind="ExternalInput")
    b2 = nc.dram_tensor("b2", [1, NF], BF16, kind="ExternalInput")
    v1 = nc.dram_tensor("v1", [EMB, EMB], BF16, kind="ExternalInput")
    c1 = nc.dram_tensor("c1", [EMB, 1], F32, kind="ExternalInput")
    v2 = nc.dram_tensor("v2", [EMB, VF], BF16, kind="ExternalInput")
    c2 = nc.dram_tensor("c2", [1, VF], BF16, kind="ExternalInput")
    a0 = nc.dram_tensor("a0", [NHO, HRI], BF16, kind="ExternalInput")
    out = nc.dram_tensor("out", [P, C * HRI], BF16, kind="ExternalOutput")

    av_dram = nc.dram_tensor("av_scratch", [P * ROWW], BF16)

    with ExitStack() as ctx:
        tc = ctx.enter_context(tile.TileContext(nc))
        cpool = ctx.enter_context(tc.tile_pool(name="consts", bufs=1))
        wpool = ctx.enter_context(tc.tile_pool(name="weights", bufs=1))
        xpool = ctx.enter_context(tc.tile_pool(name="xstream", bufs=2))
        hpool = ctx.enter_context(tc.tile_pool(name="hidden", bufs=1))
        w2pool = ctx.enter_context(tc.tile_pool(name="w2stream", bufs=2))
        l1ps = ctx.enter_context(tc.tile_pool(name="l1ps", bufs=2, space="PSUM"))
        l2ps = ctx.enter_context(tc.tile_pool(name="l2ps", bufs=TPQ, space="PSUM"))
        vps = ctx.enter_context(tc.tile_pool(name="vps", bufs=1, space="PSUM"))
        blkpool = ctx.enter_context(tc.tile_pool(name="blk", bufs=TPQ))
        pwpool = ctx.enter_context(tc.tile_pool(name="pw", bufs=2))
        atpool = ctx.enter_context(tc.tile_pool(name="at", bufs=2))
        smpool = ctx.enter_context(tc.tile_pool(name="small", bufs=3))
        agpool = ctx.enter_context(tc.tile_pool(name="agather", bufs=2))
        mopool = ctx.enter_context(tc.tile_pool(name="multout", bufs=3))
        srpool = ctx.enter_context(tc.tile_pool(name="sred", bufs=3))
        scpool = ctx.enter_context(tc.tile_pool(name="scan", bufs=1))

        # ---- constants / weights ----
        ones_s = cpool.tile([1, P], BF16, tag="ones")
        nc.vector.memset(ones_s[:], 1.0)
        b1_s = cpool.tile([P, HID // P], F32, tag="b1")
        nc.sync.dma_start(b1_s[:], b1[:].rearrange("(m p) one -> p (m one)", p=P))
        c1_s = cpool.tile([P, EMB // P], F32, tag="c1")
        nc.sync.dma_start(c1_s[:], c1[:].rearrange("(m p) one -> p (m one)", p=P))
        b2_s = cpool.tile([1, NF], BF16, tag="b2")
        nc.sync.dma_start(b2_s[:], b2[:])
        c2_s = cpool.tile([1, VF], BF16, tag="c2")
        nc.sync.dma_start(c2_s[:], c2[:])
        a0_s = cpool.tile([NHO, HRI], BF16, tag="a0")
        nc.sync.dma_start(a0_s[:], a0[:])

        w1_s = wpool.tile([P, 4, HID], BF16, tag="w1")
        nc.sync.dma_start(
            w1_s[:], bass.AP(w1, 0, [[HID, P], [P * HID, 4], [1, HID]]))
        v1_s = wpool.tile([P, 4, EMB], BF16, tag="v1")
        nc.sync.dma_start(v1_s[:], v1[:].rearrange("(k p) m -> p k m", p=P))
        v2_s = wpool.tile([P, 4, VF], BF16, tag="v2")
        nc.sync.dma_start(v2_s[:], v2[:].rearrange("(k p) n -> p k n", p=P))

        # ================= scan state =================
        NS = W + C
        s_all = scpool.tile([P, NS * HRI], BF16, tag="sall")
        s0 = scpool.tile([P, HRI], BF16, tag="s0")
        nc.vector.memset(s0[:], 0.0)

        agv_box = {}

        def gather_group(p_first):
            """One DMA fetching 8 steps' [A|v] into [P, 8*AVW]."""
            agv = agpool.tile([P, 8 * AVW], BF16, tag="agv", name=f"agv{p_first}")
            if p_first < 0:
                pos = C + p_first
                # dummy chunk-0 partitions (state discarded via a0 below)
                nc.sync.dma_start(
                    agv[0:NHO, :],
                    bass.AP(av_dram, pos * AVW, [[ROWW, NHO], [1, 8 * AVW]]))
                nc.sync.dma_start(
                    agv[NHO:P, :],
                    bass.AP(av_dram, pos * AVW,
                            [[NHO * ROWW, K - 1], [ROWW, NHO], [1, 8 * AVW]]))
            else:
                nc.sync.dma_start(
                    agv[:], bass.AP(av_dram, p_first * AVW,
                                    [[ROWW, P], [1, 8 * AVW]]))
            agv_box[p_first] = agv

        def scan_step(p):
            w = W + p
            p_first = p - (p + W) % 8
            if (p + W) % 8 == 0:
                gather_group(p_first)
            agv = agv_box[p_first]
            off = ((p + W) % 8) * AVW

            if p == -W:
                sprev_t, sprev_off = s0, 0
            else:
                sprev_t, sprev_off = s_all, (w - 1) * HRI
            # mo[(c,ho), (hr, i, k)] = A[i, k] * s_prev[hr, k]
            mo = mopool.tile([P, HL * BD], BF16, tag="mo", name=f"mo{p}")
            nc.vector.tensor_tensor(
                bass.AP(mo.tensor, mo[:].offset, [[HL * BD, P], [1, HL * BD]]),
                bass.AP(agv.tensor, agv[:].offset + off, [[8 * AVW, P], [1, HL * BD]]),
                bass.AP(sprev_t.tensor, sprev_t[:].offset + sprev_off,
                        [[sprev_t.shape[1], P], [BD, NHR], [0, BD], [1, BD]]),
                ALU.mult)
            sr = srpool.tile([P, HRI], BF16, tag="sr", name=f"sr{p}")
            with nc.allow_low_precision(reason="scan state in bf16"):
                nc.vector.tensor_reduce(
                    bass.AP(sr.tensor, sr[:].offset, [[HRI, P], [1, HRI]]),
                    bass.AP(mo.tensor, mo[:].offset,
                            [[HL * BD, P], [BD, HRI], [1, BD]]),
                    axis=mybir.AxisListType.X, op=ALU.add)
            nc.vector.tensor_tensor(
                bass.AP(s_all.tensor, s_all[:].offset + w * HRI,
                        [[NS * HRI, P], [1, HRI]]),
                bass.AP(sr.tensor, sr[:].offset, [[HRI, P], [1, HRI]]),
                bass.AP(agv.tensor, agv[:].offset + off + NHO * BD * BD,
                        [[8 * AVW, P], [1, HRI]]),
                ALU.add)
            if p == -1:
                # chunk 0 starts exactly from a0 (no warm-up): overwrite its
                # slot W-1 state after the last warm-up step wrote it.
                nc.vector.tensor_copy(s_all[0:NHO, (W - 1) * HRI:W * HRI], a0_s[:])

        def emit_out(g):
            nc.sync.dma_start(
                bass.AP(out, g * 32 * HRI, [[C * HRI, P], [1, 32 * HRI]]),
                bass.AP(s_all.tensor, s_all[:].offset + (W + 32 * g) * HRI,
                        [[NS * HRI, P], [1, 32 * HRI]]))

        # steps emitted after each q: q0 -> p in [-W, 16); q1 -> [16, 48);
        # q2 -> [48, 80); q3 -> [80, 128) (positions >= 112 use q0's tiles).
        windows = [(-W, 16), (16, 48), (48, 80), (80, C)]

        # ================= stage A =================
        for q in range(NQ):
            xq = xpool.tile([P, 4, QT], BF16, tag="xq")
            nc.sync.dma_start(
                xq[:], bass.AP(xs, q * QT,
                               [[NQ * QT, P], [P * NQ * QT, 4], [1, QT]]))

            hid_t = hpool.tile([P, HID // P, QT], BF16, tag="hid")
            for m in range(HID // P):
                ps = l1ps.tile([P, QT], F32, tag="l1")
                for k in range(4):
                    nc.tensor.matmul(ps[:], w1_s[:, k, bass.ts(m, P)], xq[:, k, :],
                                     start=(k == 0), stop=(k == 3))
                nc.scalar.activation(hid_t[:, m, :], ps[:], AF.Relu,
                                     bias=b1_s[:, m:m + 1])

            hv_t = hpool.tile([P, 4, QT], BF16, tag="hv")
            for m in range(4):
                ps = l1ps.tile([P, QT], F32, tag="l1")
                for k in range(4):
                    nc.tensor.matmul(ps[:], v1_s[:, k, bass.ts(m, P)], xq[:, k, :],
                                     start=(k == 0), stop=(k == 3))
                nc.scalar.activation(hv_t[:, m, :], ps[:], AF.Relu,
                                     bias=c1_s[:, m:m + 1])

            # ---- L2: token-major blk; W2 (q,n)-slab in 4 big DMAs ----
            blks = [blkpool.tile([P, NF], BF16, tag="blk", name=f"blk{q}_{i}")
                    for i in range(TPQ)]
            for n in range(NF // 512):
                halves = []
                for hf in range(2):
                    w2n = w2pool.tile([P, HID // (2 * P), 512], BF16, tag="w2n",
                                      name=f"w2n{q}_{n}_{hf}")
                    for g4 in range(2):
                        nc.sync.dma_start(
                            w2n[:, bass.ds(8 * g4, 8), :],
                            bass.AP(w2, (n * HID + (16 * hf + 8 * g4) * P) * 512,
                                    [[512, P], [P * 512, 8], [1, 512]]))
                    halves.append(w2n)
                pss = [l2ps.tile([P, 512], F32, tag="l2", name=f"l2ps{q}_{n}_{i}")
                       for i in range(TPQ)]
                for ttq in range(TPQ):
                    nc.tensor.matmul(pss[ttq][:], ones_s[:1, :],
                                     b2_s[:1, bass.ts(n, 512)], start=True, stop=False)
                for k in range(HID // P):
                    for ttq in range(TPQ):
                        nc.tensor.matmul(pss[ttq][:], hid_t[:, k, bass.ts(ttq, P)],
                                         halves[k // 16][:, k % 16, :], start=False,
                                         stop=(k == HID // P - 1))
                for ttq in range(TPQ):
                    nc.scalar.activation(blks[ttq][:, bass.ts(n, 512)], pss[ttq][:],
                                         AF.Identity)

            # ---- v2 + normalization per tok-tile ----
            for ttq in range(TPQ):
                tau = q * TPQ + ttq
                at = atpool.tile([P, NHO * AVW], BF16, tag="at", name=f"at{q}_{ttq}")

                psv = vps.tile([P, VF], F32, tag="v")
                nc.tensor.matmul(psv[:], ones_s[:1, :], c2_s[:1, :],
                                 start=True, stop=False)
                for k in range(4):
                    nc.tensor.matmul(psv[:], hv_t[:, k, bass.ts(ttq, P)],
                                     v2_s[:, k, :], start=False, stop=(k == 3))
                nc.scalar.activation(
                    bass.AP(at.tensor, at[:].offset + NHO * BD * BD,
                            [[NHO * AVW, P], [AVW, NHO], [1, HRI]]),
                    bass.AP(psv.tensor, psv[:].offset, [[VF, P], [HRI, NHO], [1, HRI]]),
                    AF.Identity)

                blk = blks[ttq]
                # |blk|^1.2 = exp(0.6 * ln(blk^2)); feats are (h, i, k)
                pw = pwpool.tile([P, NF], BF16, tag="pw")
                nc.scalar.activation(pw[:], blk[:], AF.Square)
                nc.scalar.activation(pw[:], pw[:], AF.Ln)
                nc.scalar.activation(pw[:], pw[:], AF.Exp, scale=0.6)
                # sum over rows i -> pst[(h, k)]
                pst = smpool.tile([P, HL * BD], F32, tag="pst")
                with nc.allow_low_precision(reason="norm stats"):
                    nc.vector.tensor_reduce(
                        pst[:].rearrange("p (h k) -> p h k", h=HL, k=BD),
                        bass.AP(pw.tensor, pw[:].offset,
                                [[NF, P], [64, HL], [1, BD], [8, BD]]),
                        axis=mybir.AxisListType.X, op=ALU.add)
                # max_k commutes with ^(1/1.2); rc = dm^(-1/1.2)
                dm = smpool.tile([P, HL], F32, tag="dm")
                nc.vector.tensor_reduce(
                    dm[:].rearrange("p (h one) -> p h one", h=HL, one=1),
                    pst[:].rearrange("p (h k) -> p h k", h=HL, k=BD),
                    axis=mybir.AxisListType.X, op=ALU.max)
                nc.scalar.activation(dm[:], dm[:], AF.Ln)
                rch = smpool.tile([P, HL], BF16, tag="rch")
                nc.scalar.activation(rch[:], dm[:], AF.Exp, scale=-1.0 / 1.2)

                # A = blk * rc (broadcast over i, k), (h, i, k) order
                nc.vector.tensor_tensor(
                    bass.AP(at.tensor, at[:].offset,
                            [[NHO * AVW, P], [AVW, NHO], [64, NHR], [1, 64]]),
                    bass.AP(blk.tensor, blk[:].offset,
                            [[NF, P], [NHR * 64, NHO], [64, NHR], [1, 64]]),
                    bass.AP(rch.tensor, rch[:].offset,
                            [[HL, P], [NHR, NHO], [1, NHR], [0, 64]]),
                    ALU.mult)
                for ho in range(NHO):
                    nc.sync.dma_start(
                        bass.AP(av_dram, ho * ROWW + _rot(tau) * AVW,
                                [[NHO * ROWW, K], [AVW, 8], [1, AVW]]),
                        bass.AP(at.tensor, at[:].offset + ho * AVW,
                                [[NHO * AVW, P], [1, AVW]]))

            # ---- scan steps unlocked by this q ----
            lo, hi = windows[q]
            for p in range(lo, hi):
                if scan_steps is not None and p - (-W) >= scan_steps:
                    continue
                scan_step(p)
                if p + 1 in (32, 64, 96):
                    emit_out(p // 32)
            if q == NQ - 1:
                emit_out(3)

    nc.compile()
    return nc


# ---------------- host side ----------------

_NC_CACHE = {}


def _get_nc(TOK=SEQ):
    if TOK not in _NC_CACHE:
        _NC_CACHE[TOK] = build_nc(TOK=TOK)
    return _NC_CACHE[TOK]


def _stripe_tokens():
    """token index for MLP column (tau, c, j) order, flattened [NQ*QT]."""
    cols = np.zeros(SEQ, np.int64)
    i = 0
    for tau in range(SEQ // P):
        for c in range(K):
            for j in range(8):
                cols[i] = c * C + _rot(tau) + j
                i += 1
    return cols


def prep_shared(W1, b1, W2, b2, V1, c1, V2, c2, a0):
    bf = ml_dtypes.bfloat16
    W2r = W2.reshape(H, BD, BD, HID)
    W2c = (W2r - W2r.mean(axis=1, keepdims=True)).reshape(H * BD * BD, HID)
    b2r = b2.reshape(H, BD, BD)
    b2c = (b2r - b2r.mean(axis=1, keepdims=True)).reshape(-1)
    shared = {
        "w1": np.ascontiguousarray(W1.T).astype(bf),
        "b1": np.asarray(b1).reshape(HID, 1).astype(np.float32),
        "v1": np.ascontiguousarray(V1.T).astype(bf),
        "c1": np.asarray(c1).reshape(EMB, 1).astype(np.float32),
    }
    halves = []
    for half in range(2):
        rsl = slice(half * NF, (half + 1) * NF)
        vsl = slice(half * VF, (half + 1) * VF)
        hsl = slice(half * HL, (half + 1) * HL)
        a0h = np.asarray(a0)[0, hsl]                       # [32, 8]
        a0p = a0h.reshape(NHO, NHR, BD).reshape(NHO, HRI)  # [ho, (hr, i)]
        w2h = np.ascontiguousarray(W2c[rsl].T).astype(bf)  # [HID, NF]
        w2n = np.ascontiguousarray(
            w2h.reshape(HID, NF // 512, 512).transpose(1, 0, 2)).reshape(-1)
        halves.append({
            "w2": w2n,
            "b2": b2c[rsl].reshape(1, NF).astype(bf),
            "v2": np.ascontiguousarray(V2[vsl].T).astype(bf),
            "c2": np.asarray(c2)[vsl].reshape(1, VF).astype(bf),
            "a0": a0p.astype(bf),
        })
    return shared, halves


def make_in_maps(x, W1, b1, W2, b2, V1, c1, V2, c2, a0):
    shared, halves = prep_shared(W1, b1, W2, b2, V1, c1, V2, c2, a0)
    bf = ml_dtypes.bfloat16
    cols = _stripe_tokens()
    in_maps = []
    for core in range(N_CORES):
        b, half = core // 2, core % 2
        m = dict(shared)
        m.update(halves[half])
        xT = np.asarray(x)[b].T.astype(bf)            # [EMB, SEQ]
        xst = xT[:, cols]                             # striped columns
        # xs[k, p, q, col]
        m["xs"] = np.ascontiguousarray(
            xst.reshape(4, P, NQ_G, QT_G)).reshape(-1)
        in_maps.append(m)
    return in_maps


NQ_G = SEQ // 512
QT_G = 512


def kernel(x, W1, b1, W2, b2, V1, c1, V2, c2, a0):
    from concourse import bass_utils
    nc = _get_nc(SEQ)
    in_maps = make_in_maps(x, W1, b1, W2, b2, V1, c1, V2, c2, a0)
    res = bass_utils.run_bass_kernel_spmd(nc, in_maps, core_ids=list(range(N_CORES)))
    out = np.zeros((BS, SEQ, EMB), np.float32)
    for core in range(N_CORES):
        b, half = core // 2, core % 2
        raw = res.results[core]["out"].astype(np.float32)   # [128, C*HRI]
        o = raw.reshape(K, NHO, C, NHR, BD).transpose(0, 2, 1, 3, 4)
        out[b, :, half * VF:(half + 1) * VF] = o.reshape(SEQ, VF)
    return out


# revision 15
# speedup vs baseline: 1.6352x; 1.0328x over previous
"""Trainium2 Bass kernel for nn_BlockModel_82678120448388.

Model: per (batch, head): 8x8 transition matrices from an MLP (normalized),
values from a second MLP, then a linear recurrence s_t = A_t s_{t-1} + v_t
over seq=2048.

Sharding: 8 cores = 4 batches x 2 head-halves (32 heads each). Weights
replicated / row-sliced on host; full inputs in, full output out.

Scan strategy: the normalized A_t are strongly contractive (product over a
16-token window has norm ~1e-5), so the recurrence is chunk-local to far
below the error tolerance. Each core runs K=16 independent chunk scans of
C=128 tokens in partition-parallel, each warmed up with the last W=16
tokens of the previous chunk from a zero state; chunk 0 starts exactly
from a0.

DMA layout: A and v for token (chunk c, pos p) are stored contiguously in
av_dram[(c,ho), p, 288] so one DMA gathers 8 scan steps; W2 is host-
re-laid-out so each (q, n) slab loads in 4 large DMAs; x is host-striped
so each q loads in one DMA.
"""

import numpy as np
import ml_dtypes
from contextlib import ExitStack

import concourse.bass as bass
import concourse.bacc as bacc
import concourse.tile as tile
from concourse import mybir

F32 = mybir.dt.float32
BF16 = mybir.dt.bfloat16
AF = mybir.ActivationFunctionType
ALU = mybir.AluOpType

BS, SEQ, EMB, BD = 4, 2048, 512, 8
H = EMB // BD      # 64 global heads
HL = 32            # heads per core
NF = HL * BD * BD  # 2048 blk feats per core
VF = HL * BD       # 256 v feats per core
HID = EMB * BD     # 4096
P = 128

N_CORES = 8

K = 16             # chunks per core
C = SEQ // K       # 128 tokens per chunk
W = 16             # warm-up tokens per chunk
NHO = P // K       # 8 head-groups on partitions
NHR = HL // NHO    # 4 heads per group in free dim
HRI = NHR * BD     # 32
AVW = NHO * BD * BD + BD * NHR  # 288: [A 256 | v 32] per (c,ho,pos)
ROWW = C * AVW     # av_dram row size per (c, ho)


def _rot(tau):
    """Within-chunk position of the first token in MLP tile tau.

    Warm-up positions [112, 128) are produced by tiles 0-1 so the scan's
    warm-up steps only depend on the first q's MLP output.
    """
    return (112 + 8 * tau) % C


def _tau_of(pos):
    return (pos - 112) // 8 if pos >= 112 else pos // 8 + 2


def build_nc(TOK=SEQ, scan_steps=None):
    QT = 512
    NQ = TOK // QT
    TPQ = QT // P

    nc = bacc.Bacc("TRN2", target_bir_lowering=False, debug=False)

    # xs[k, p, q, col]: pre-striped x so each q loads in one DMA
    xs = nc.dram_tensor("xs", [4 * P * NQ * QT], BF16, kind="ExternalInput")
    w1 = nc.dram_tensor("w1", [EMB, HID], BF16, kind="ExternalInput")
    b1 = nc.dram_tensor("b1", [HID, 1], F32, kind="ExternalInput")
    # w2n[n, hid, f]: per-n contiguous slabs
    w2 = nc.dram_tensor("w2", [(NF // 512) * HID * 512], BF16, kind="ExternalInput")
    b2 = nc.dram_tensor("b2", [1, NF], BF16, kind="ExternalInput")
    v1 = nc.dram_tensor("v1", [EMB, EMB], BF16, kind="ExternalInput")
    c1 = nc.dram_tensor("c1", [EMB, 1], F32, kind="ExternalInput")
    v2 = nc.dram_tensor("v2", [EMB, VF], BF16, kind="ExternalInput")
    c2 = nc.dram_tensor("c2", [1, VF], BF16, kind="ExternalInput")
    a0 = nc.dram_tensor("a0", [NHO, HRI], BF16, kind="ExternalInput")
    out = nc.dram_tensor("out", [P, C * HRI], BF16, kind="ExternalOutput")

    av_dram = nc.dram_tensor("av_scratch", [P * ROWW], BF16)

    with ExitStack() as ctx:
        tc = ctx.enter_context(tile.TileContext(nc))
        cpool = ctx.enter_context(tc.tile_pool(name="consts", bufs=1))
        wpool = ctx.enter_context(tc.tile_pool(name="weights", bufs=1))
        xpool = ctx.enter_context(tc.tile_pool(name="xstream", bufs=2))
        hpool = ctx.enter_context(tc.tile_pool(name="hidden", bufs=1))
        w2pool = ctx.enter_context(tc.tile_pool(name="w2stream", bufs=2))
        l1ps = ctx.enter_context(tc.tile_pool(name="l1ps", bufs=2, space="PSUM"))
        l2ps = ctx.enter_context(tc.tile_pool(name="l2ps", bufs=TPQ, space="PSUM"))
        vps = ctx.enter_context(tc.tile_pool(name="vps", bufs=1, space="PSUM"))
        blkpool = ctx.enter_context(tc.tile_pool(name="blk", bufs=TPQ))
        pwpool = ctx.enter_context(tc.tile_pool(name="pw", bufs=2))
        atpool = ctx.enter_context(tc.tile_pool(name="at", bufs=4))
        smpool = ctx.enter_context(tc.tile_pool(name="small", bufs=4))
        agpool = ctx.enter_context(tc.tile_pool(name="agather", bufs=2))
        mopool = ctx.enter_context(tc.tile_pool(name="multout", bufs=2))
        srpool = ctx.enter_context(tc.tile_pool(name="sred", bufs=2))
        scpool = ctx.enter_context(tc.tile_pool(name="scan", bufs=1))

        # ---- constants / weights ----
        ones_s = cpool.tile([1, P], BF16, tag="ones")
        nc.vector.memset(ones_s[:], 1.0)
        b1_s = cpool.tile([P, HID // P], F32, tag="b1")
        nc.sync.dma_start(b1_s[:], b1[:].rearrange("(m p) one -> p (m one)", p=P))
        c1_s = cpool.tile([P, EMB // P], F32, tag="c1")
        nc.sync.dma_start(c1_s[:], c1[:].rearrange("(m p) one -> p (m one)", p=P))
        b2_s = cpool.tile([1, NF], BF16, tag="b2")
        nc.sync.dma_start(b2_s[:], b2[:])
        c2_s = cpool.tile([1, VF], BF16, tag="c2")
        nc.sync.dma_start(c2_s[:], c2[:])
        a0_s = cpool.tile([NHO, HRI], BF16, tag="a0")
        nc.sync.dma_start(a0_s[:], a0[:])

        w1_s = wpool.tile([P, 4, HID], BF16, tag="w1")
        for g4 in range(4):
            nc.sync.dma_start(
                w1_s[:, :, bass.ds(g4 * HID // 4, HID // 4)],
                bass.AP(w1, g4 * HID // 4,
                        [[HID, P], [P * HID, 4], [1, HID // 4]]))
        v1_s = wpool.tile([P, 4, EMB], BF16, tag="v1")
        nc.sync.dma_start(v1_s[:], v1[:].rearrange("(k p) m -> p k m", p=P))
        v2_s = wpool.tile([P, 4, VF], BF16, tag="v2")
        nc.sync.dma_start(v2_s[:], v2[:].rearrange("(k p) n -> p k n", p=P))

        # ================= scan state =================
        NS = W + C
        s_all = scpool.tile([P, NS * HRI], BF16, tag="sall")
        s0 = scpool.tile([P, HRI], BF16, tag="s0")
        nc.vector.memset(s0[:], 0.0)

        agv_box = {}

        def gather_group(p_first):
            """One DMA fetching 8 steps' [A|v] into [P, 8*AVW]."""
            agv = agpool.tile([P, 8 * AVW], BF16, tag="agv", name=f"agv{p_first}")
            if p_first < 0:
                pos = C + p_first
                # dummy chunk-0 partitions (state discarded via a0 below)
                nc.sync.dma_start(
                    agv[0:NHO, :],
                    bass.AP(av_dram, pos * AVW, [[ROWW, NHO], [1, 8 * AVW]]))
                nc.sync.dma_start(
                    agv[NHO:P, :],
                    bass.AP(av_dram, pos * AVW,
                            [[NHO * ROWW, K - 1], [ROWW, NHO], [1, 8 * AVW]]))
            else:
                nc.sync.dma_start(
                    agv[:], bass.AP(av_dram, p_first * AVW,
                                    [[ROWW, P], [1, 8 * AVW]]))
            agv_box[p_first] = agv

        def scan_step(p):
            w = W + p
            p_first = p - (p + W) % 8
            if (p + W) % 8 == 0:
                gather_group(p_first)
            agv = agv_box[p_first]
            off = ((p + W) % 8) * AVW

            if p == -W:
                sprev_t, sprev_off = s0, 0
            else:
                sprev_t, sprev_off = s_all, (w - 1) * HRI
            # mo[(c,ho), (hr, i, k)] = A[i, k] * s_prev[hr, k]
            mo = mopool.tile([P, HL * BD], BF16, tag="mo", name=f"mo{p}")
            nc.vector.tensor_tensor(
                bass.AP(mo.tensor, mo[:].offset, [[HL * BD, P], [1, HL * BD]]),
                bass.AP(agv.tensor, agv[:].offset + off, [[8 * AVW, P], [1, HL * BD]]),
                bass.AP(sprev_t.tensor, sprev_t[:].offset + sprev_off,
                        [[sprev_t.shape[1], P], [BD, NHR], [0, BD], [1, BD]]),
                ALU.mult)
            sr = srpool.tile([P, HRI], BF16, tag="sr", name=f"sr{p}")
            with nc.allow_low_precision(reason="scan state in bf16"):
                nc.vector.tensor_reduce(
                    bass.AP(sr.tensor, sr[:].offset, [[HRI, P], [1, HRI]]),
                    bass.AP(mo.tensor, mo[:].offset,
                            [[HL * BD, P], [BD, HRI], [1, BD]]),
                    axis=mybir.AxisListType.X, op=ALU.add)
            nc.vector.tensor_tensor(
                bass.AP(s_all.tensor, s_all[:].offset + w * HRI,
                        [[NS * HRI, P], [1, HRI]]),
                bass.AP(sr.tensor, sr[:].offset, [[HRI, P], [1, HRI]]),
                bass.AP(agv.tensor, agv[:].offset + off + NHO * BD * BD,
                        [[8 * AVW, P], [1, HRI]]),
                ALU.add)
            if p == -1:
                # chunk 0 starts exactly from a0 (no warm-up): overwrite its
                # slot W-1 state after the last warm-up step wrote it.
                nc.vector.tensor_copy(s_all[0:NHO, (W - 1) * HRI:W * HRI], a0_s[:])

        def emit_out(g):
            nc.sync.dma_start(
                bass.AP(out, g * 32 * HRI, [[C * HRI, P], [1, 32 * HRI]]),
                bass.AP(s_all.tensor, s_all[:].offset + (W + 32 * g) * HRI,
                        [[NS * HRI, P], [1, 32 * HRI]]))

        # steps emitted after each q: q0 -> p in [-W, 16); q1 -> [16, 48);
        # q2 -> [48, 80); q3 -> [80, 128) (positions >= 112 use q0's tiles).
        windows = [(-W, 16), (16, 48), (48, 80), (80, C)]

        # ================= stage A =================
        for q in range(NQ):
            xq = xpool.tile([P, 4, QT], BF16, tag="xq")
            nc.sync.dma_start(
                xq[:], bass.AP(xs, q * QT,
                               [[NQ * QT, P], [P * NQ * QT, 4], [1, QT]]))

            hid_t = hpool.tile([P, HID // P, QT], BF16, tag="hid")
            for m in range(HID // P):
                ps = l1ps.tile([P, QT], F32, tag="l1")
                for k in range(4):
                    nc.tensor.matmul(ps[:], w1_s[:, k, bass.ts(m, P)], xq[:, k, :],
                                     start=(k == 0), stop=(k == 3))
                nc.scalar.activation(hid_t[:, m, :], ps[:], AF.Relu,
                                     bias=b1_s[:, m:m + 1])

            hv_t = hpool.tile([P, 4, QT], BF16, tag="hv")
            for m in range(4):
                ps = l1ps.tile([P, QT], F32, tag="l1")
                for k in range(4):
                    nc.tensor.matmul(ps[:], v1_s[:, k, bass.ts(m, P)], xq[:, k, :],
                                     start=(k == 0), stop=(k == 3))
                nc.scalar.activation(hv_t[:, m, :], ps[:], AF.Relu,
                                     bias=c1_s[:, m:m + 1])

            # ---- L2: token-major blk; W2 (q,n)-slab in 4 big DMAs ----
            blks = [blkpool.tile([P, NF], BF16, tag="blk", name=f"blk{q}_{i}")
                    for i in range(TPQ)]
            for n in range(NF // 512):
                pss = [l2ps.tile([P, 512], F32, tag="l2", name=f"l2ps{q}_{n}_{i}")
                       for i in range(TPQ)]
                for ttq in range(TPQ):
                    nc.tensor.matmul(pss[ttq][:], ones_s[:1, :],
                                     b2_s[:1, bass.ts(n, 512)], start=True, stop=False)
                for qtr in range(4):
                    w2q = w2pool.tile([P, 8, 512], BF16, tag="w2n",
                                      name=f"w2n{q}_{n}_{qtr}")
                    nc.sync.dma_start(
                        w2q[:], bass.AP(w2, (n * HID + 8 * qtr * P) * 512,
                                        [[512, P], [P * 512, 8], [1, 512]]))
                    for k8 in range(8):
                        k = qtr * 8 + k8
                        for ttq in range(TPQ):
                            nc.tensor.matmul(pss[ttq][:], hid_t[:, k, bass.ts(ttq, P)],
                                             w2q[:, k8, :], start=False,
                                             stop=(k == HID // P - 1))
                for ttq in range(TPQ):
                    nc.scalar.activation(blks[ttq][:, bass.ts(n, 512)], pss[ttq][:],
                                         AF.Identity)

            # ---- v2 + normalization, acts batched per function ----
            ats = [atpool.tile([P, NHO * AVW], BF16, tag="at", name=f"at{q}_{i}")
                   for i in range(TPQ)]
            for ttq in range(TPQ):
                psv = vps.tile([P, VF], F32, tag="v")
                nc.tensor.matmul(psv[:], ones_s[:1, :], c2_s[:1, :],
                                 start=True, stop=False)
                for k in range(4):
                    nc.tensor.matmul(psv[:], hv_t[:, k, bass.ts(ttq, P)],
                                     v2_s[:, k, :], start=False, stop=(k == 3))
                nc.scalar.activation(
                    bass.AP(ats[ttq].tensor, ats[ttq][:].offset + NHO * BD * BD,
                            [[NHO * AVW, P], [AVW, NHO], [1, HRI]]),
                    bass.AP(psv.tensor, psv[:].offset, [[VF, P], [HRI, NHO], [1, HRI]]),
                    AF.Identity)
            # |blk|^1.2 = exp(0.6 * ln(blk^2)); square on DVE, feats (h, i, k)
            rchs = []
            for hf in range(2):
                pws = [pwpool.tile([P, NF], BF16, tag="pw", name=f"pw{q}_{hf}_{i}")
                       for i in range(2)]
                for i, pw in enumerate(pws):
                    nc.vector.tensor_tensor(pw[:], blks[2 * hf + i][:],
                                            blks[2 * hf + i][:], ALU.mult)
                for pw in pws:
                    nc.scalar.activation(pw[:], pw[:], AF.Ln)
                for pw in pws:
                    nc.scalar.activation(pw[:], pw[:], AF.Exp, scale=0.6)
                for i, pw in enumerate(pws):
                    # sum over rows i -> pst[(h, k)]
                    pst = smpool.tile([P, HL * BD], F32, tag="pst")
                    with nc.allow_low_precision(reason="norm stats"):
                        nc.vector.tensor_reduce(
                            pst[:].rearrange("p (h k) -> p h k", h=HL, k=BD),
                            bass.AP(pw.tensor, pw[:].offset,
                                    [[NF, P], [64, HL], [1, BD], [8, BD]]),
                            axis=mybir.AxisListType.X, op=ALU.add)
                    # max_k commutes with ^(1/1.2); rc = dm^(-1/1.2)
                    dm = smpool.tile([P, HL], F32, tag="dm",
                                     name=f"dm{q}_{2 * hf + i}")
                    nc.vector.tensor_reduce(
                        dm[:].rearrange("p (h one) -> p h one", h=HL, one=1),
                        pst[:].rearrange("p (h k) -> p h k", h=HL, k=BD),
                        axis=mybir.AxisListType.X, op=ALU.max)
                    rchs.append(dm)
            for dm in rchs:
                nc.scalar.activation(dm[:], dm[:], AF.Ln)
            rcbs = []
            for ttq, dm in enumerate(rchs):
                rch = smpool.tile([P, HL], BF16, tag="rch", name=f"rch{q}_{ttq}")
                nc.scalar.activation(rch[:], dm[:], AF.Exp, scale=-1.0 / 1.2)
                rcbs.append(rch)
            for ttq in range(TPQ):
                tau = q * TPQ + ttq
                at = ats[ttq]
                # A = blk * rc (broadcast over i, k), (h, i, k) order
                nc.vector.tensor_tensor(
                    bass.AP(at.tensor, at[:].offset,
                            [[NHO * AVW, P], [AVW, NHO], [64, NHR], [1, 64]]),
                    bass.AP(blks[ttq].tensor, blks[ttq][:].offset,
                            [[NF, P], [NHR * 64, NHO], [64, NHR], [1, 64]]),
                    bass.AP(rcbs[ttq].tensor, rcbs[ttq][:].offset,
                            [[HL, P], [NHR, NHO], [1, NHR], [0, 64]]),
                    ALU.mult)
                for ho in range(NHO):
                    nc.sync.dma_start(
                        bass.AP(av_dram, ho * ROWW + _rot(tau) * AVW,
                                [[NHO * ROWW, K], [AVW, 8], [1, AVW]]),
                        bass.AP(at.tensor, at[:].offset + ho * AVW,
                                [[NHO * AVW, P], [1, AVW]]))

            # ---- scan steps unlocked by this q ----
            lo, hi = windows[q]
            for p in range(lo, hi):
                if scan_steps is not None and p - (-W) >= scan_steps:
                    continue
                scan_step(p)
                if p + 1 in (32, 64, 96):
                    emit_out(p // 32)
            if q == NQ - 1:
                emit_out(3)

    nc.compile()
    return nc


# ---------------- host side ----------------

_NC_CACHE = {}


def _get_nc(TOK=SEQ):
    if TOK not in _NC_CACHE:
        _NC_CACHE[TOK] = build_nc(TOK=TOK)
    return _NC_CACHE[TOK]


def _stripe_tokens():
    """token index for MLP column (tau, c, j) order, flattened [NQ*QT]."""
    cols = np.zeros(SEQ, np.int64)
    i = 0
    for tau in range(SEQ // P):
        for c in range(K):
            for j in range(8):
                cols[i] = c * C + _rot(tau) + j
                i += 1
    return cols


def prep_shared(W1, b1, W2, b2, V1, c1, V2, c2, a0):
    bf = ml_dtypes.bfloat16
    W2r = W2.reshape(H, BD, BD, HID)
    W2c = (W2r - W2r.mean(axis=1, keepdims=True)).reshape(H * BD * BD, HID)
    b2r = b2.reshape(H, BD, BD)
    b2c = (b2r - b2r.mean(axis=1, keepdims=True)).reshape(-1)
    shared = {
        "w1": np.ascontiguousarray(W1.T).astype(bf),
        "b1": np.asarray(b1).reshape(HID, 1).astype(np.float32),
        "v1": np.ascontiguousarray(V1.T).astype(bf),
        "c1": np.asarray(c1).reshape(EMB, 1).astype(np.float32),
    }
    halves = []
    for half in range(2):
        rsl = slice(half * NF, (half + 1) * NF)
        vsl = slice(half * VF, (half + 1) * VF)
        hsl = slice(half * HL, (half + 1) * HL)
        a0h = np.asarray(a0)[0, hsl]                       # [32, 8]
        a0p = a0h.reshape(NHO, NHR, BD).reshape(NHO, HRI)  # [ho, (hr, i)]
        w2h = np.ascontiguousarray(W2c[rsl].T).astype(bf)  # [HID, NF]
        w2n = np.ascontiguousarray(
            w2h.reshape(HID, NF // 512, 512).transpose(1, 0, 2)).reshape(-1)
        halves.append({
            "w2": w2n,
            "b2": b2c[rsl].reshape(1, NF).astype(bf),
            "v2": np.ascontiguousarray(V2[vsl].T).astype(bf),
            "c2": np.asarray(c2)[vsl].reshape(1, VF).astype(bf),
            "a0": a0p.astype(bf),
        })
    return shared, halves


def make_in_maps(x, W1, b1, W2, b2, V1, c1, V2, c2, a0):
    shared, halves = prep_shared(W1, b1, W2, b2, V1, c1, V2, c2, a0)
    bf = ml_dtypes.bfloat16
    cols = _stripe_tokens()
    in_maps = []
    for core in range(N_CORES):
        b, half = core // 2, core % 2
        m = dict(shared)
        m.update(halves[half])
        xT = np.asarray(x)[b].T.astype(bf)            # [EMB, SEQ]
        xst = xT[:, cols]                             # striped columns
        # xs[k, p, q, col]
        m["xs"] = np.ascontiguousarray(
            xst.reshape(4, P, NQ_G, QT_G)).reshape(-1)
        in_maps.append(m)
    return in_maps


NQ_G = SEQ // 512
QT_G = 512


def kernel(x, W1, b1, W2, b2, V1, c1, V2, c2, a0):
    from concourse import bass_utils
    nc = _get_nc(SEQ)
    in_maps = make_in_maps(x, W1, b1, W2, b2, V1, c1, V2, c2, a0)
    res = bass_utils.run_bass_kernel_spmd(nc, in_maps, core_ids=list(range(N_CORES)))
    out = np.zeros((BS, SEQ, EMB), np.float32)
    for core in range(N_CORES):
        b, half = core // 2, core % 2
        raw = res.results[core]["out"].astype(np.float32)   # [128, C*HRI]
        o = raw.reshape(K, NHO, C, NHR, BD).transpose(0, 2, 1, 3, 4)
        out[b, :, half * VF:(half + 1) * VF] = o.reshape(SEQ, VF)
    return out


# revision 22
# speedup vs baseline: 1.6646x; 1.0180x over previous
"""Trainium2 Bass kernel for nn_BlockModel_82678120448388.

Model: per (batch, head): 8x8 transition matrices from an MLP (normalized),
values from a second MLP, then a linear recurrence s_t = A_t s_{t-1} + v_t
over seq=2048.

Sharding: 8 cores = 4 batches x 2 head-halves (32 heads each). Weights
replicated / row-sliced on host; full inputs in, full output out.

Scan strategy: the normalized A_t are strongly contractive (product over a
16-token window has norm ~1e-5), so the recurrence is chunk-local to far
below the error tolerance. Each core runs K=16 independent chunk scans of
C=128 tokens in partition-parallel, each warmed up with the last W=16
tokens of the previous chunk from a zero state; chunk 0 starts exactly
from a0.

DMA layout: A and v for token (chunk c, pos p) are stored contiguously in
av_dram[(c,ho), p, 288] so one DMA gathers 8 scan steps; W2 is host-
re-laid-out so each (q, n) slab loads in 4 large DMAs; x is host-striped
so each q loads in one DMA.
"""

import numpy as np
import ml_dtypes
from contextlib import ExitStack

import concourse.bass as bass
import concourse.bacc as bacc
import concourse.tile as tile
from concourse import mybir

F32 = mybir.dt.float32
BF16 = mybir.dt.bfloat16
AF = mybir.ActivationFunctionType
ALU = mybir.AluOpType

BS, SEQ, EMB, BD = 4, 2048, 512, 8
H = EMB // BD      # 64 global heads
HL = 32            # heads per core
NF = HL * BD * BD  # 2048 blk feats per core
VF = HL * BD       # 256 v feats per core
HID = EMB * BD     # 4096
P = 128

N_CORES = 8

K = 16             # chunks per core
C = SEQ // K       # 128 tokens per chunk
W = 16             # warm-up tokens per chunk
NHO = P // K       # 8 head-groups on partitions
NHR = HL // NHO    # 4 heads per group in free dim
HRI = NHR * BD     # 32
AVW = NHR * BD * 9  # 288: per (c,ho,pos): (hr, i, [A row | v]) 9-wide rows
ROWW = C * AVW     # av_dram row size per (c, ho)


def _rot(tau):
    """Within-chunk position of the first token in MLP tile tau.

    Warm-up positions [112, 128) are produced by tiles 0-1 so the scan's
    warm-up steps only depend on the first q's MLP output.
    """
    return (112 + 8 * tau) % C


def _tau_of(pos):
    return (pos - 112) // 8 if pos >= 112 else pos // 8 + 2


def build_nc(TOK=SEQ, scan_steps=None):
    QT = 512
    NQ = TOK // QT
    TPQ = QT // P

    nc = bacc.Bacc("TRN2", target_bir_lowering=False, debug=False)

    # xs[k, p, q, col]: pre-striped x so each q loads in one DMA
    xs = nc.dram_tensor("xs", [4 * P * NQ * QT], BF16, kind="ExternalInput")
    w1 = nc.dram_tensor("w1", [EMB, HID], BF16, kind="ExternalInput")
    b1 = nc.dram_tensor("b1", [HID, 1], F32, kind="ExternalInput")
    # w2n[n, hid, f]: per-n contiguous slabs
    w2 = nc.dram_tensor("w2", [(NF // 512) * HID * 512], BF16, kind="ExternalInput")
    b2 = nc.dram_tensor("b2", [1, NF], BF16, kind="ExternalInput")
    v1 = nc.dram_tensor("v1", [EMB, EMB], BF16, kind="ExternalInput")
    c1 = nc.dram_tensor("c1", [EMB, 1], F32, kind="ExternalInput")
    v2 = nc.dram_tensor("v2", [EMB, VF], BF16, kind="ExternalInput")
    c2 = nc.dram_tensor("c2", [1, VF], BF16, kind="ExternalInput")
    a0 = nc.dram_tensor("a0", [NHO, HRI], BF16, kind="ExternalInput")
    out = nc.dram_tensor("out", [P, C * HRI], BF16, kind="ExternalOutput")

    av_dram = nc.dram_tensor("av_scratch", [P * ROWW], BF16)

    with ExitStack() as ctx:
        tc = ctx.enter_context(tile.TileContext(nc))
        cpool = ctx.enter_context(tc.tile_pool(name="consts", bufs=1))
        wpool = ctx.enter_context(tc.tile_pool(name="weights", bufs=1))
        xpool = ctx.enter_context(tc.tile_pool(name="xstream", bufs=2))
        hpool = ctx.enter_context(tc.tile_pool(name="hidden", bufs=1))
        w2pool = ctx.enter_context(tc.tile_pool(name="w2stream", bufs=2))
        l1ps = ctx.enter_context(tc.tile_pool(name="l1ps", bufs=2, space="PSUM"))
        l2ps = ctx.enter_context(tc.tile_pool(name="l2ps", bufs=TPQ, space="PSUM"))
        vps = ctx.enter_context(tc.tile_pool(name="vps", bufs=1, space="PSUM"))
        blkpool = ctx.enter_context(tc.tile_pool(name="blk", bufs=TPQ))
        pwpool = ctx.enter_context(tc.tile_pool(name="pw", bufs=4))
        atpool = ctx.enter_context(tc.tile_pool(name="at", bufs=4))
        smpool = ctx.enter_context(tc.tile_pool(name="small", bufs=4))
        agpool = ctx.enter_context(tc.tile_pool(name="agather", bufs=2))
        mopool = ctx.enter_context(tc.tile_pool(name="multout", bufs=2))
        srpool = ctx.enter_context(tc.tile_pool(name="sred", bufs=2))
        scpool = ctx.enter_context(tc.tile_pool(name="scan", bufs=1))

        # ---- constants / weights ----
        ones_s = cpool.tile([1, P], BF16, tag="ones")
        nc.vector.memset(ones_s[:], 1.0)
        b1_s = cpool.tile([P, HID // P], F32, tag="b1")
        nc.sync.dma_start(b1_s[:], b1[:].rearrange("(m p) one -> p (m one)", p=P))
        c1_s = cpool.tile([P, EMB // P], F32, tag="c1")
        nc.sync.dma_start(c1_s[:], c1[:].rearrange("(m p) one -> p (m one)", p=P))
        b2_s = cpool.tile([1, NF], BF16, tag="b2")
        nc.sync.dma_start(b2_s[:], b2[:])
        c2_s = cpool.tile([1, VF], BF16, tag="c2")
        nc.sync.dma_start(c2_s[:], c2[:])
        a0_s = cpool.tile([NHO, HRI], BF16, tag="a0")
        nc.sync.dma_start(a0_s[:], a0[:])

        v1_s = wpool.tile([P, 4, EMB], BF16, tag="v1")
        nc.sync.dma_start(v1_s[:], v1[:].rearrange("(k p) m -> p k m", p=P))
        v2_s = wpool.tile([P, 4, VF], BF16, tag="v2")
        nc.sync.dma_start(v2_s[:], v2[:].rearrange("(k p) n -> p k n", p=P))

        # ================= scan state =================
        # s_all slot w (36 per hr-group... 4*9=36 wide): cols hr*9+i = state,
        # col hr*9+8 = constant 1.0 so the fused step's 9-wide reduce adds v.
        NS = W + C
        SW = NS * NHR * 9
        s_all = scpool.tile([P, SW], BF16, tag="sall")
        nc.vector.memset(
            bass.AP(s_all.tensor, s_all[:].offset + 8, [[SW, P], [9, NS * NHR]]),
            1.0)
        s0 = scpool.tile([P, NHR * 9], BF16, tag="s0")
        nc.vector.memset(s0[:], 0.0)
        nc.vector.memset(
            bass.AP(s0.tensor, s0[:].offset + 8, [[NHR * 9, P], [9, NHR]]), 1.0)

        agv_box = {}

        def gather_group(p_first):
            """One DMA fetching 8 steps' [A|v] into [P, 8*AVW]."""
            agv = agpool.tile([P, 8 * AVW], BF16, tag="agv", name=f"agv{p_first}")
            if p_first < 0:
                pos = C + p_first
                # dummy chunk-0 partitions (state discarded via a0 below)
                nc.sync.dma_start(
                    agv[0:NHO, :],
                    bass.AP(av_dram, pos * AVW, [[ROWW, NHO], [1, 8 * AVW]]))
                nc.sync.dma_start(
                    agv[NHO:P, :],
                    bass.AP(av_dram, pos * AVW,
                            [[NHO * ROWW, K - 1], [ROWW, NHO], [1, 8 * AVW]]))
            else:
                nc.sync.dma_start(
                    agv[:], bass.AP(av_dram, p_first * AVW,
                                    [[ROWW, P], [1, 8 * AVW]]))
            agv_box[p_first] = agv

        def scan_step(p):
            w = W + p
            p_first = p - (p + W) % 8
            if (p + W) % 8 == 0:
                gather_group(p_first)
            agv = agv_box[p_first]
            off = ((p + W) % 8) * AVW

            if p == -W:
                sprev_t, sprev_off = s0, 0
            else:
                sprev_t, sprev_off = s_all, (w - 1) * NHR * 9
            # mo[(c,ho), (hr, i, 9)] = [A|v][i, :] * [s_prev[hr, :] | 1]
            mo = mopool.tile([P, AVW], BF16, tag="mo", name=f"mo{p}")
            nc.vector.tensor_tensor(
                bass.AP(mo.tensor, mo[:].offset, [[AVW, P], [1, AVW]]),
                bass.AP(agv.tensor, agv[:].offset + off, [[8 * AVW, P], [1, AVW]]),
                bass.AP(sprev_t.tensor, sprev_t[:].offset + sprev_off,
                        [[sprev_t.shape[1], P], [9, NHR], [0, BD], [1, 9]]),
                ALU.mult)
            with nc.allow_low_precision(reason="scan state in bf16"):
                nc.vector.tensor_reduce(
                    bass.AP(s_all.tensor, s_all[:].offset + w * NHR * 9,
                            [[SW, P], [9, NHR], [1, BD]]),
                    bass.AP(mo.tensor, mo[:].offset,
                            [[AVW, P], [72, NHR], [9, BD], [1, 9]]),
                    axis=mybir.AxisListType.X, op=ALU.add)
            if p == -1:
                # chunk 0 starts exactly from a0 (no warm-up): overwrite its
                # slot W-1 state after the last warm-up step wrote it.
                nc.vector.tensor_copy(
                    bass.AP(s_all.tensor, s_all[0:NHO, :].offset + (W - 1) * NHR * 9,
                            [[SW, NHO], [9, NHR], [1, BD]]),
                    bass.AP(a0_s.tensor, a0_s[:].offset,
                            [[HRI, NHO], [BD, NHR], [1, BD]]))

        def emit_out(g):
            # positions [32g, 32g+32): one DMA per hr (strided 9-wide slots)
            for hr in range(NHR):
                nc.sync.dma_start(
                    bass.AP(out, g * 32 * HRI + hr * BD,
                            [[C * HRI, P], [HRI, 32], [1, BD]]),
                    bass.AP(s_all.tensor,
                            s_all[:].offset + (W + 32 * g) * NHR * 9 + hr * 9,
                            [[SW, P], [NHR * 9, 32], [1, BD]]))

        # steps emitted after each q: q0 -> p in [-W, 16); q1 -> [16, 48);
        # q2 -> [48, 80); q3 -> [80, 128) (positions >= 112 use q0's tiles).
        windows = [(-W, 16), (16, 48), (48, 80), (80, C)]

        # ================= stage A =================
        for q in range(NQ):
            xq = xpool.tile([P, 4, QT], BF16, tag="xq")
            nc.sync.dma_start(
                xq[:], bass.AP(xs, q * QT,
                               [[NQ * QT, P], [P * NQ * QT, 4], [1, QT]]))

            hid_t = hpool.tile([P, HID // P, QT], BF16, tag="hid")
            for m in range(HID // P):
                if m % 8 == 0:
                    w1q = w2pool.tile([P, 4, HID // 4], BF16, tag="w1q",
                                      name=f"w1q{q}_{m // 8}")
                    nc.sync.dma_start(
                        w1q[:], bass.AP(w1, (m // 8) * (HID // 4),
                                        [[HID, P], [P * HID, 4], [1, HID // 4]]))
                ps = l1ps.tile([P, QT], F32, tag="l1")
                for k in range(4):
                    nc.tensor.matmul(ps[:], w1q[:, k, bass.ts(m % 8, P)], xq[:, k, :],
                                     start=(k == 0), stop=(k == 3))
                nc.scalar.activation(hid_t[:, m, :], ps[:], AF.Relu,
                                     bias=b1_s[:, m:m + 1])

            hv_t = hpool.tile([P, 4, QT], BF16, tag="hv")
            for m in range(4):
                ps = l1ps.tile([P, QT], F32, tag="l1")
                for k in range(4):
                    nc.tensor.matmul(ps[:], v1_s[:, k, bass.ts(m, P)], xq[:, k, :],
                                     start=(k == 0), stop=(k == 3))
                nc.scalar.activation(hv_t[:, m, :], ps[:], AF.Relu,
                                     bias=c1_s[:, m:m + 1])

            # ---- L2: token-major blk; W2 (q,n)-slab in 4 big DMAs ----
            blks = [blkpool.tile([P, NF], BF16, tag="blk", name=f"blk{q}_{i}")
                    for i in range(TPQ)]
            for n in range(NF // 512):
                pss = [l2ps.tile([P, 512], F32, tag="l2", name=f"l2ps{q}_{n}_{i}")
                       for i in range(TPQ)]
                for ttq in range(TPQ):
                    nc.tensor.matmul(pss[ttq][:], ones_s[:1, :],
                                     b2_s[:1, bass.ts(n, 512)], start=True, stop=False)
                for qtr in range(4):
                    w2q = w2pool.tile([P, 8, 512], BF16, tag="w2n",
                                      name=f"w2n{q}_{n}_{qtr}")
                    nc.sync.dma_start(
                        w2q[:], bass.AP(w2, (n * HID + 8 * qtr * P) * 512,
                                        [[512, P], [P * 512, 8], [1, 512]]))
                    for k8 in range(8):
                        k = qtr * 8 + k8
                        for ttq in range(TPQ):
                            nc.tensor.matmul(pss[ttq][:], hid_t[:, k, bass.ts(ttq, P)],
                                             w2q[:, k8, :], start=False,
                                             stop=(k == HID // P - 1))
                for ttq in range(TPQ):
                    nc.scalar.activation(blks[ttq][:, bass.ts(n, 512)], pss[ttq][:],
                                         AF.Identity)

            # ---- v2 + normalization, acts batched per function ----
            # at layout per (c,j) partition: (ho, hr, i, 9) with [A cols | v]
            ats = [atpool.tile([P, NHO * AVW], BF16, tag="at", name=f"at{q}_{i}")
                   for i in range(TPQ)]
            for ttq in range(TPQ):
                psv = vps.tile([P, VF], F32, tag="v")
                nc.tensor.matmul(psv[:], ones_s[:1, :], c2_s[:1, :],
                                 start=True, stop=False)
                for k in range(4):
                    nc.tensor.matmul(psv[:], hv_t[:, k, bass.ts(ttq, P)],
                                     v2_s[:, k, :], start=False, stop=(k == 3))
                nc.scalar.activation(
                    bass.AP(ats[ttq].tensor, ats[ttq][:].offset + 8,
                            [[NHO * AVW, P], [72, HL], [9, BD]]),
                    bass.AP(psv.tensor, psv[:].offset, [[VF, P], [8, HL], [1, BD]]),
                    AF.Identity)
            # |blk|^1.2 = exp(0.6 * ln(blk^2)); square on DVE, feats (h, i, k)
            pws = [pwpool.tile([P, NF], BF16, tag="pw", name=f"pw{q}_{i}")
                   for i in range(TPQ)]
            for ttq in range(TPQ):
                nc.vector.tensor_tensor(pws[ttq][:], blks[ttq][:], blks[ttq][:],
                                        ALU.mult)
            for pw in pws:
                nc.scalar.activation(pw[:], pw[:], AF.Ln)
            for pw in pws:
                nc.scalar.activation(pw[:], pw[:], AF.Exp, scale=0.6)
            rchs = []
            for ttq in range(TPQ):
                # sum over rows i -> pst[(h, k)]
                pst = smpool.tile([P, HL * BD], F32, tag="pst")
                with nc.allow_low_precision(reason="norm stats"):
                    nc.vector.tensor_reduce(
                        pst[:].rearrange("p (h k) -> p h k", h=HL, k=BD),
                        bass.AP(pws[ttq].tensor, pws[ttq][:].offset,
                                [[NF, P], [64, HL], [1, BD], [8, BD]]),
                        axis=mybir.AxisListType.X, op=ALU.add)
                # max_k commutes with ^(1/1.2); rc = dm^(-1/1.2)
                dm = smpool.tile([P, HL], F32, tag="dm", name=f"dm{q}_{ttq}")
                nc.vector.tensor_reduce(
                    dm[:].rearrange("p (h one) -> p h one", h=HL, one=1),
                    pst[:].rearrange("p (h k) -> p h k", h=HL, k=BD),
                    axis=mybir.AxisListType.X, op=ALU.max)
                rchs.append(dm)
            for dm in rchs:
                nc.scalar.activation(dm[:], dm[:], AF.Ln)
            rcbs = []
            for ttq, dm in enumerate(rchs):
                rch = smpool.tile([P, HL], BF16, tag="rch", name=f"rch{q}_{ttq}")
                nc.scalar.activation(rch[:], dm[:], AF.Exp, scale=-1.0 / 1.2)
                rcbs.append(rch)
            for ttq in range(TPQ):
                tau = q * TPQ + ttq
                at = ats[ttq]
                # A = blk * rc (broadcast over i, k) into 9-strided at slots
                nc.vector.tensor_tensor(
                    bass.AP(at.tensor, at[:].offset,
                            [[NHO * AVW, P], [72, HL], [9, BD], [1, BD]]),
                    bass.AP(blks[ttq].tensor, blks[ttq][:].offset,
                            [[NF, P], [64, HL], [8, BD], [1, BD]]),
                    bass.AP(rcbs[ttq].tensor, rcbs[ttq][:].offset,
                            [[HL, P], [1, HL], [0, BD], [0, BD]]),
                    ALU.mult)
                for ho in range(NHO):
                    nc.sync.dma_start(
                        bass.AP(av_dram, ho * ROWW + _rot(tau) * AVW,
                                [[NHO * ROWW, K], [AVW, 8], [1, AVW]]),
                        bass.AP(at.tensor, at[:].offset + ho * AVW,
                                [[NHO * AVW, P], [1, AVW]]))

            # ---- scan steps unlocked by this q ----
            lo, hi = windows[q]
            for p in range(lo, hi):
                if scan_steps is not None and p - (-W) >= scan_steps:
                    continue
                scan_step(p)
                if p + 1 in (32, 64, 96):
                    emit_out(p // 32)
            if q == NQ - 1:
                emit_out(3)

    nc.compile()
    return nc


# ---------------- host side ----------------

_NC_CACHE = {}


def _get_nc(TOK=SEQ):
    if TOK not in _NC_CACHE:
        _NC_CACHE[TOK] = build_nc(TOK=TOK)
    return _NC_CACHE[TOK]


def _stripe_tokens():
    """token index for MLP column (tau, c, j) order, flattened [NQ*QT]."""
    cols = np.zeros(SEQ, np.int64)
    i = 0
    for tau in range(SEQ // P):
        for c in range(K):
            for j in range(8):
                cols[i] = c * C + _rot(tau) + j
                i += 1
    return cols


def prep_shared(W1, b1, W2, b2, V1, c1, V2, c2, a0):
    bf = ml_dtypes.bfloat16
    W2r = W2.reshape(H, BD, BD, HID)
    W2c = (W2r - W2r.mean(axis=1, keepdims=True)).reshape(H * BD * BD, HID)
    b2r = b2.reshape(H, BD, BD)
    b2c = (b2r - b2r.mean(axis=1, keepdims=True)).reshape(-1)
    shared = {
        "w1": np.ascontiguousarray(W1.T).astype(bf),
        "b1": np.asarray(b1).reshape(HID, 1).astype(np.float32),
        "v1": np.ascontiguousarray(V1.T).astype(bf),
        "c1": np.asarray(c1).reshape(EMB, 1).astype(np.float32),
    }
    halves = []
    for half in range(2):
        rsl = slice(half * NF, (half + 1) * NF)
        vsl = slice(half * VF, (half + 1) * VF)
        hsl = slice(half * HL, (half + 1) * HL)
        a0h = np.asarray(a0)[0, hsl]                       # [32, 8]
        a0p = a0h.reshape(NHO, NHR, BD).reshape(NHO, HRI)  # [ho, (hr, i)]
        w2h = np.ascontiguousarray(W2c[rsl].T).astype(bf)  # [HID, NF]
        w2n = np.ascontiguousarray(
            w2h.reshape(HID, NF // 512, 512).transpose(1, 0, 2)).reshape(-1)
        halves.append({
            "w2": w2n,
            "b2": b2c[rsl].reshape(1, NF).astype(bf),
            "v2": np.ascontiguousarray(V2[vsl].T).astype(bf),
            "c2": np.asarray(c2)[vsl].reshape(1, VF).astype(bf),
            "a0": a0p.astype(bf),
        })
    return shared, halves


def make_in_maps(x, W1, b1, W2, b2, V1, c1, V2, c2, a0):
    shared, halves = prep_shared(W1, b1, W2, b2, V1, c1, V2, c2, a0)
    bf = ml_dtypes.bfloat16
    cols = _stripe_tokens()
    in_maps = []
    for core in range(N_CORES):
        b, half = core // 2, core % 2
        m = dict(shared)
        m.update(halves[half])
        xT = np.asarray(x)[b].T.astype(bf)            # [EMB, SEQ]
        xst = xT[:, cols]                             # striped columns
        # xs[k, p, q, col]
        m["xs"] = np.ascontiguousarray(
            xst.reshape(4, P, NQ_G, QT_G)).reshape(-1)
        in_maps.append(m)
    return in_maps


NQ_G = SEQ // 512
QT_G = 512


def kernel(x, W1, b1, W2, b2, V1, c1, V2, c2, a0):
    from concourse import bass_utils
    nc = _get_nc(SEQ)
    in_maps = make_in_maps(x, W1, b1, W2, b2, V1, c1, V2, c2, a0)
    res = bass_utils.run_bass_kernel_spmd(nc, in_maps, core_ids=list(range(N_CORES)))
    out = np.zeros((BS, SEQ, EMB), np.float32)
    for core in range(N_CORES):
        b, half = core // 2, core % 2
        raw = res.results[core]["out"].astype(np.float32)   # [128, C*HRI]
        o = raw.reshape(K, NHO, C, NHR, BD).transpose(0, 2, 1, 3, 4)
        out[b, :, half * VF:(half + 1) * VF] = o.reshape(SEQ, VF)
    return out


# revision 24
# speedup vs baseline: 1.7986x; 1.0805x over previous
"""Trainium2 Bass kernel for nn_BlockModel_82678120448388.

Model: per (batch, head): 8x8 transition matrices from an MLP (normalized),
values from a second MLP, then a linear recurrence s_t = A_t s_{t-1} + v_t
over seq=2048.

Sharding: 8 cores = 4 batches x 2 head-halves (32 heads each). Weights
replicated / row-sliced on host; full inputs in, full output out.

Scan strategy: the normalized A_t are strongly contractive (product over a
16-token window has norm ~1e-5), so the recurrence is chunk-local to far
below the error tolerance. Each core runs K=16 independent chunk scans of
C=128 tokens in partition-parallel, each warmed up with the last W=16
tokens of the previous chunk from a zero state; chunk 0 starts exactly
from a0.

DMA layout: A and v for token (chunk c, pos p) are stored contiguously in
av_dram[(c,ho), p, 288] so one DMA gathers 8 scan steps; W2 is host-
re-laid-out so each (q, n) slab loads in 4 large DMAs; x is host-striped
so each q loads in one DMA.
"""

import numpy as np
import ml_dtypes
from contextlib import ExitStack

import concourse.bass as bass
import concourse.bacc as bacc
import concourse.tile as tile
from concourse import mybir

F32 = mybir.dt.float32
BF16 = mybir.dt.bfloat16
AF = mybir.ActivationFunctionType
ALU = mybir.AluOpType

BS, SEQ, EMB, BD = 4, 2048, 512, 8
H = EMB // BD      # 64 global heads
HL = 32            # heads per core
NF = HL * BD * BD  # 2048 blk feats per core
VF = HL * BD       # 256 v feats per core
HID = EMB * BD     # 4096
P = 128

N_CORES = 8

K = 16             # chunks per core
C = SEQ // K       # 128 tokens per chunk
W = 16             # warm-up tokens per chunk
NHO = P // K       # 8 head-groups on partitions
NHR = HL // NHO    # 4 heads per group in free dim
HRI = NHR * BD     # 32
AVW = NHR * BD * 9  # 288: per (c,ho,pos): (hr, i, [A row | v]) 9-wide rows
ROWW = C * AVW     # av_dram row size per (c, ho)


def _rot(tau):
    """Within-chunk position of the first token in MLP tile tau.

    Warm-up positions [112, 128) are produced by tiles 0-1 so the scan's
    warm-up steps only depend on the first q's MLP output.
    """
    return (112 + 8 * tau) % C


def _tau_of(pos):
    return (pos - 112) // 8 if pos >= 112 else pos // 8 + 2


def build_nc(TOK=SEQ, scan_steps=None):
    QT = 512
    NQ = TOK // QT
    TPQ = QT // P

    nc = bacc.Bacc("TRN2", target_bir_lowering=False, debug=False)

    # xs[k, p, q, col]: pre-striped x so each q loads in one DMA
    xs = nc.dram_tensor("xs", [4 * P * NQ * QT], BF16, kind="ExternalInput")
    w1 = nc.dram_tensor("w1", [EMB, HID], BF16, kind="ExternalInput")
    b1 = nc.dram_tensor("b1", [HID, 1], F32, kind="ExternalInput")
    # w2n[n, hid, f]: per-n contiguous slabs
    w2 = nc.dram_tensor("w2", [(NF // 512) * HID * 512], BF16, kind="ExternalInput")
    b2 = nc.dram_tensor("b2", [1, NF], BF16, kind="ExternalInput")
    v1 = nc.dram_tensor("v1", [EMB, EMB], BF16, kind="ExternalInput")
    c1 = nc.dram_tensor("c1", [EMB, 1], F32, kind="ExternalInput")
    v2 = nc.dram_tensor("v2", [EMB, VF], BF16, kind="ExternalInput")
    c2 = nc.dram_tensor("c2", [1, VF], BF16, kind="ExternalInput")
    a0 = nc.dram_tensor("a0", [NHO, HRI], BF16, kind="ExternalInput")
    out = nc.dram_tensor("out", [P, C * HRI], BF16, kind="ExternalOutput")

    av_dram = nc.dram_tensor("av_scratch", [P * ROWW], BF16)

    with ExitStack() as ctx:
        tc = ctx.enter_context(tile.TileContext(nc))
        cpool = ctx.enter_context(tc.tile_pool(name="consts", bufs=1))
        wpool = ctx.enter_context(tc.tile_pool(name="weights", bufs=1))
        xpool = ctx.enter_context(tc.tile_pool(name="xstream", bufs=2))
        hpool = ctx.enter_context(tc.tile_pool(name="hidden", bufs=1))
        hvpool = ctx.enter_context(tc.tile_pool(name="hv", bufs=2))
        w2pool = ctx.enter_context(tc.tile_pool(name="w2stream", bufs=2))
        l1ps = ctx.enter_context(tc.tile_pool(name="l1ps", bufs=2, space="PSUM"))
        l2ps = ctx.enter_context(tc.tile_pool(name="l2ps", bufs=TPQ, space="PSUM"))
        vps = ctx.enter_context(tc.tile_pool(name="vps", bufs=1, space="PSUM"))
        blkpool = ctx.enter_context(tc.tile_pool(name="blk", bufs=2 * TPQ))
        pwpool = ctx.enter_context(tc.tile_pool(name="pw", bufs=4))
        atpool = ctx.enter_context(tc.tile_pool(name="at", bufs=4))
        smpool = ctx.enter_context(tc.tile_pool(name="small", bufs=4))
        agpool = ctx.enter_context(tc.tile_pool(name="agather", bufs=2))
        mopool = ctx.enter_context(tc.tile_pool(name="multout", bufs=2))
        srpool = ctx.enter_context(tc.tile_pool(name="sred", bufs=2))
        scpool = ctx.enter_context(tc.tile_pool(name="scan", bufs=1))

        # ---- constants / weights ----
        ones_s = cpool.tile([1, P], BF16, tag="ones")
        nc.vector.memset(ones_s[:], 1.0)
        b1_s = cpool.tile([P, HID // P], F32, tag="b1")
        nc.sync.dma_start(b1_s[:], b1[:].rearrange("(m p) one -> p (m one)", p=P))
        c1_s = cpool.tile([P, EMB // P], F32, tag="c1")
        nc.sync.dma_start(c1_s[:], c1[:].rearrange("(m p) one -> p (m one)", p=P))
        b2_s = cpool.tile([1, NF], BF16, tag="b2")
        nc.sync.dma_start(b2_s[:], b2[:])
        c2_s = cpool.tile([1, VF], BF16, tag="c2")
        nc.sync.dma_start(c2_s[:], c2[:])
        a0_s = cpool.tile([NHO, HRI], BF16, tag="a0")
        nc.sync.dma_start(a0_s[:], a0[:])

        v1_s = wpool.tile([P, 4, EMB], BF16, tag="v1")
        nc.sync.dma_start(v1_s[:], v1[:].rearrange("(k p) m -> p k m", p=P))
        v2_s = wpool.tile([P, 4, VF], BF16, tag="v2")
        nc.sync.dma_start(v2_s[:], v2[:].rearrange("(k p) n -> p k n", p=P))

        # ================= scan state =================
        # s_all slot w (36 per hr-group... 4*9=36 wide): cols hr*9+i = state,
        # col hr*9+8 = constant 1.0 so the fused step's 9-wide reduce adds v.
        NS = W + C
        SW = NS * NHR * 9
        s_all = scpool.tile([P, SW], BF16, tag="sall")
        nc.vector.memset(
            bass.AP(s_all.tensor, s_all[:].offset + 8, [[SW, P], [9, NS * NHR]]),
            1.0)
        s0 = scpool.tile([P, NHR * 9], BF16, tag="s0")
        nc.vector.memset(s0[:], 0.0)
        nc.vector.memset(
            bass.AP(s0.tensor, s0[:].offset + 8, [[NHR * 9, P], [9, NHR]]), 1.0)

        agv_box = {}

        def gather_group(p_first):
            """One DMA fetching 8 steps' [A|v] into [P, 8*AVW]."""
            agv = agpool.tile([P, 8 * AVW], BF16, tag="agv", name=f"agv{p_first}")
            if p_first < 0:
                pos = C + p_first
                # dummy chunk-0 partitions (state discarded via a0 below)
                nc.sync.dma_start(
                    agv[0:NHO, :],
                    bass.AP(av_dram, pos * AVW, [[ROWW, NHO], [1, 8 * AVW]]))
                nc.sync.dma_start(
                    agv[NHO:P, :],
                    bass.AP(av_dram, pos * AVW,
                            [[NHO * ROWW, K - 1], [ROWW, NHO], [1, 8 * AVW]]))
            else:
                nc.sync.dma_start(
                    agv[:], bass.AP(av_dram, p_first * AVW,
                                    [[ROWW, P], [1, 8 * AVW]]))
            agv_box[p_first] = agv

        def scan_step(p):
            w = W + p
            p_first = p - (p + W) % 8
            if (p + W) % 8 == 0:
                gather_group(p_first)
            agv = agv_box[p_first]
            off = ((p + W) % 8) * AVW

            if p == -W:
                sprev_t, sprev_off = s0, 0
            else:
                sprev_t, sprev_off = s_all, (w - 1) * NHR * 9
            # mo[(c,ho), (hr, i, 9)] = [A|v][i, :] * [s_prev[hr, :] | 1]
            mo = mopool.tile([P, AVW], BF16, tag="mo", name=f"mo{p}")
            nc.vector.tensor_tensor(
                bass.AP(mo.tensor, mo[:].offset, [[AVW, P], [1, AVW]]),
                bass.AP(agv.tensor, agv[:].offset + off, [[8 * AVW, P], [1, AVW]]),
                bass.AP(sprev_t.tensor, sprev_t[:].offset + sprev_off,
                        [[sprev_t.shape[1], P], [9, NHR], [0, BD], [1, 9]]),
                ALU.mult)
            with nc.allow_low_precision(reason="scan state in bf16"):
                nc.vector.tensor_reduce(
                    bass.AP(s_all.tensor, s_all[:].offset + w * NHR * 9,
                            [[SW, P], [9, NHR], [1, BD]]),
                    bass.AP(mo.tensor, mo[:].offset,
                            [[AVW, P], [72, NHR], [9, BD], [1, 9]]),
                    axis=mybir.AxisListType.X, op=ALU.add)
            if p == -1:
                # chunk 0 starts exactly from a0 (no warm-up): overwrite its
                # slot W-1 state after the last warm-up step wrote it.
                nc.vector.tensor_copy(
                    bass.AP(s_all.tensor, s_all[0:NHO, :].offset + (W - 1) * NHR * 9,
                            [[SW, NHO], [9, NHR], [1, BD]]),
                    bass.AP(a0_s.tensor, a0_s[:].offset,
                            [[HRI, NHO], [BD, NHR], [1, BD]]))

        def emit_out(g):
            # positions [32g, 32g+32): one DMA per hr (strided 9-wide slots)
            for hr in range(NHR):
                nc.sync.dma_start(
                    bass.AP(out, g * 32 * HRI + hr * BD,
                            [[C * HRI, P], [HRI, 32], [1, BD]]),
                    bass.AP(s_all.tensor,
                            s_all[:].offset + (W + 32 * g) * NHR * 9 + hr * 9,
                            [[SW, P], [NHR * 9, 32], [1, BD]]))

        # steps unlocked per producing q: q0 -> p in [-W, 16); q1 -> [16, 48);
        # q2 -> [48, 80); q3 -> [80, 128) (positions >= 112 use q0's tiles).
        windows = [(-W, 16), (16, 48), (48, 80), (80, C)]

        def emit_part1(q):
            """x load + L1 + v-hidden (PE + Act relu)."""
            xq = xpool.tile([P, 4, QT], BF16, tag="xq")
            nc.sync.dma_start(
                xq[:], bass.AP(xs, q * QT,
                               [[NQ * QT, P], [P * NQ * QT, 4], [1, QT]]))
            hid_t = hpool.tile([P, HID // P, QT], BF16, tag="hid")
            for m in range(HID // P):
                if m % 8 == 0:
                    w1q = w2pool.tile([P, 4, HID // 4], BF16, tag="w1q",
                                      name=f"w1q{q}_{m // 8}")
                    nc.sync.dma_start(
                        w1q[:], bass.AP(w1, (m // 8) * (HID // 4),
                                        [[HID, P], [P * HID, 4], [1, HID // 4]]))
                ps = l1ps.tile([P, QT], F32, tag="l1")
                for k in range(4):
                    nc.tensor.matmul(ps[:], w1q[:, k, bass.ts(m % 8, P)], xq[:, k, :],
                                     start=(k == 0), stop=(k == 3))
                nc.scalar.activation(hid_t[:, m, :], ps[:], AF.Relu,
                                     bias=b1_s[:, m:m + 1])
            hv_t = hvpool.tile([P, 4, QT], BF16, tag="hv", name=f"hv{q}")
            for m in range(4):
                ps = l1ps.tile([P, QT], F32, tag="l1")
                for k in range(4):
                    nc.tensor.matmul(ps[:], v1_s[:, k, bass.ts(m, P)], xq[:, k, :],
                                     start=(k == 0), stop=(k == 3))
                nc.scalar.activation(hv_t[:, m, :], ps[:], AF.Relu,
                                     bias=c1_s[:, m:m + 1])
            return {"q": q, "hid": hid_t, "hv": hv_t}

        def emit_l2(st):
            """L2: token-major blk; W2 streamed in quarter slabs."""
            q, hid_t = st["q"], st["hid"]
            blks = [blkpool.tile([P, NF], BF16, tag="blk", name=f"blk{q}_{i}")
                    for i in range(TPQ)]
            for n in range(NF // 512):
                pss = [l2ps.tile([P, 512], F32, tag="l2", name=f"l2ps{q}_{n}_{i}")
                       for i in range(TPQ)]
                for ttq in range(TPQ):
                    nc.tensor.matmul(pss[ttq][:], ones_s[:1, :],
                                     b2_s[:1, bass.ts(n, 512)], start=True, stop=False)
                for qtr in range(4):
                    w2q = w2pool.tile([P, 8, 512], BF16, tag="w2n",
                                      name=f"w2n{q}_{n}_{qtr}")
                    nc.sync.dma_start(
                        w2q[:], bass.AP(w2, (n * HID + 8 * qtr * P) * 512,
                                        [[512, P], [P * 512, 8], [1, 512]]))
                    for k8 in range(8):
                        k = qtr * 8 + k8
                        for ttq in range(TPQ):
                            nc.tensor.matmul(pss[ttq][:], hid_t[:, k, bass.ts(ttq, P)],
                                             w2q[:, k8, :], start=False,
                                             stop=(k == HID // P - 1))
                for ttq in range(TPQ):
                    nc.scalar.activation(blks[ttq][:, bass.ts(n, 512)], pss[ttq][:],
                                         AF.Identity)
            st["blks"] = blks

        def emit_vnorm(st):
            """v2 psums + v write into at tiles (PE + small act)."""
            q, hv_t = st["q"], st["hv"]
            ats = [atpool.tile([P, NHO * AVW], BF16, tag="at", name=f"at{q}_{i}")
                   for i in range(TPQ)]
            for ttq in range(TPQ):
                psv = vps.tile([P, VF], F32, tag="v")
                nc.tensor.matmul(psv[:], ones_s[:1, :], c2_s[:1, :],
                                 start=True, stop=False)
                for k in range(4):
                    nc.tensor.matmul(psv[:], hv_t[:, k, bass.ts(ttq, P)],
                                     v2_s[:, k, :], start=False, stop=(k == 3))
                nc.scalar.activation(
                    bass.AP(ats[ttq].tensor, ats[ttq][:].offset + 8,
                            [[NHO * AVW, P], [72, HL], [9, BD]]),
                    bass.AP(psv.tensor, psv[:].offset, [[VF, P], [8, HL], [1, BD]]),
                    AF.Identity)
            st["ats"] = ats

        def emit_norm(st):
            """|blk|^1.2 norm + A write-out (DVE + Act, batched)."""
            q, blks, ats = st["q"], st["blks"], st["ats"]
            pws = [pwpool.tile([P, NF], BF16, tag="pw", name=f"pw{q}_{i}")
                   for i in range(TPQ)]
            for ttq in range(TPQ):
                nc.vector.tensor_tensor(pws[ttq][:], blks[ttq][:], blks[ttq][:],
                                        ALU.mult)
            for pw in pws:
                nc.scalar.activation(pw[:], pw[:], AF.Ln)
            for pw in pws:
                nc.scalar.activation(pw[:], pw[:], AF.Exp, scale=0.6)
            rchs = []
            for ttq in range(TPQ):
                # sum over rows i -> pst[(h, k)]
                pst = smpool.tile([P, HL * BD], F32, tag="pst")
                with nc.allow_low_precision(reason="norm stats"):
                    nc.vector.tensor_reduce(
                        pst[:].rearrange("p (h k) -> p h k", h=HL, k=BD),
                        bass.AP(pws[ttq].tensor, pws[ttq][:].offset,
                                [[NF, P], [64, HL], [1, BD], [8, BD]]),
                        axis=mybir.AxisListType.X, op=ALU.add)
                # max_k commutes with ^(1/1.2); rc = dm^(-1/1.2)
                dm = smpool.tile([P, HL], F32, tag="dm", name=f"dm{q}_{ttq}")
                nc.vector.tensor_reduce(
                    dm[:].rearrange("p (h one) -> p h one", h=HL, one=1),
                    pst[:].rearrange("p (h k) -> p h k", h=HL, k=BD),
                    axis=mybir.AxisListType.X, op=ALU.max)
                rchs.append(dm)
            for dm in rchs:
                nc.scalar.activation(dm[:], dm[:], AF.Ln)
            rcbs = []
            for ttq, dm in enumerate(rchs):
                rch = smpool.tile([P, HL], BF16, tag="rch", name=f"rch{q}_{ttq}")
                nc.scalar.activation(rch[:], dm[:], AF.Exp, scale=-1.0 / 1.2)
                rcbs.append(rch)
            for ttq in range(TPQ):
                tau = q * TPQ + ttq
                at = ats[ttq]
                # A = blk * rc (broadcast over i, k) into 9-strided at slots
                nc.vector.tensor_tensor(
                    bass.AP(at.tensor, at[:].offset,
                            [[NHO * AVW, P], [72, HL], [9, BD], [1, BD]]),
                    bass.AP(blks[ttq].tensor, blks[ttq][:].offset,
                            [[NF, P], [64, HL], [8, BD], [1, BD]]),
                    bass.AP(rcbs[ttq].tensor, rcbs[ttq][:].offset,
                            [[HL, P], [1, HL], [0, BD], [0, BD]]),
                    ALU.mult)
                for ho in range(NHO):
                    nc.sync.dma_start(
                        bass.AP(av_dram, ho * ROWW + _rot(tau) * AVW,
                                [[NHO * ROWW, K], [AVW, 8], [1, AVW]]),
                        bass.AP(at.tensor, at[:].offset + ho * AVW,
                                [[NHO * AVW, P], [1, AVW]]))

        def emit_window(q):
            lo, hi = windows[q]
            for p in range(lo, hi):
                scan_step(p)
                if p + 1 in (32, 64, 96):
                    emit_out(p // 32)
            if q == NQ - 1:
                emit_out(3)

        # ======== software-pipelined emission: norm(q-1) under stage-A(q) ====
        prev = None
        for q in range(NQ):
            st = emit_part1(q)
            if prev is not None:
                emit_vnorm(prev)
            emit_l2(st)
            if prev is not None:
                emit_norm(prev)
                emit_window(prev["q"])
            prev = st
        emit_vnorm(prev)
        emit_norm(prev)
        emit_window(NQ - 1)

    nc.compile()
    return nc


# ---------------- host side ----------------

_NC_CACHE = {}


def _get_nc(TOK=SEQ):
    if TOK not in _NC_CACHE:
        _NC_CACHE[TOK] = build_nc(TOK=TOK)
    return _NC_CACHE[TOK]


def _stripe_tokens():
    """token index for MLP column (tau, c, j) order, flattened [NQ*QT]."""
    cols = np.zeros(SEQ, np.int64)
    i = 0
    for tau in range(SEQ // P):
        for c in range(K):
            for j in range(8):
                cols[i] = c * C + _rot(tau) + j
                i += 1
    return cols


def prep_shared(W1, b1, W2, b2, V1, c1, V2, c2, a0):
    bf = ml_dtypes.bfloat16
    W2r = W2.reshape(H, BD, BD, HID)
    W2c = (W2r - W2r.mean(axis=1, keepdims=True)).reshape(H * BD * BD, HID)
    b2r = b2.reshape(H, BD, BD)
    b2c = (b2r - b2r.mean(axis=1, keepdims=True)).reshape(-1)
    shared = {
        "w1": np.ascontiguousarray(W1.T).astype(bf),
        "b1": np.asarray(b1).reshape(HID, 1).astype(np.float32),
        "v1": np.ascontiguousarray(V1.T).astype(bf),
        "c1": np.asarray(c1).reshape(EMB, 1).astype(np.float32),
    }
    halves = []
    for half in range(2):
        rsl = slice(half * NF, (half + 1) * NF)
        vsl = slice(half * VF, (half + 1) * VF)
        hsl = slice(half * HL, (half + 1) * HL)
        a0h = np.asarray(a0)[0, hsl]                       # [32, 8]
        a0p = a0h.reshape(NHO, NHR, BD).reshape(NHO, HRI)  # [ho, (hr, i)]
        w2h = np.ascontiguousarray(W2c[rsl].T).astype(bf)  # [HID, NF]
        w2n = np.ascontiguousarray(
            w2h.reshape(HID, NF // 512, 512).transpose(1, 0, 2)).reshape(-1)
        halves.append({
            "w2": w2n,
            "b2": b2c[rsl].reshape(1, NF).astype(bf),
            "v2": np.ascontiguousarray(V2[vsl].T).astype(bf),
            "c2": np.asarray(c2)[vsl].reshape(1, VF).astype(bf),
            "a0": a0p.astype(bf),
        })
    return shared, halves


def make_in_maps(x, W1, b1, W2, b2, V1, c1, V2, c2, a0):
    shared, halves = prep_shared(W1, b1, W2, b2, V1, c1, V2, c2, a0)
    bf = ml_dtypes.bfloat16
    cols = _stripe_tokens()
    in_maps = []
    for core in range(N_CORES):
        b, half = core // 2, core % 2
        m = dict(shared)
        m.update(halves[half])
        xT = np.asarray(x)[b].T.astype(bf)            # [EMB, SEQ]
        xst = xT[:, cols]                             # striped columns
        # xs[k, p, q, col]
        m["xs"] = np.ascontiguousarray(
            xst.reshape(4, P, NQ_G, QT_G)).reshape(-1)
        in_maps.append(m)
    return in_maps


NQ_G = SEQ // 512
QT_G = 512


def kernel(x, W1, b1, W2, b2, V1, c1, V2, c2, a0):
    from concourse import bass_utils
    nc = _get_nc(SEQ)
    in_maps = make_in_maps(x, W1, b1, W2, b2, V1, c1, V2, c2, a0)
    res = bass_utils.run_bass_kernel_spmd(nc, in_maps, core_ids=list(range(N_CORES)))
    out = np.zeros((BS, SEQ, EMB), np.float32)
    for core in range(N_CORES):
        b, half = core // 2, core % 2
        raw = res.results[core]["out"].astype(np.float32)   # [128, C*HRI]
        o = raw.reshape(K, NHO, C, NHR, BD).transpose(0, 2, 1, 3, 4)
        out[b, :, half * VF:(half + 1) * VF] = o.reshape(SEQ, VF)
    return out
